# revision 17
# baseline (speedup 1.0000x reference)
"""Trainium2 Bass kernel for nn_EnhancedGNNTransformerEncoder (4-layer
TransformerConv GNN + mean-pool + linear head).

Sharding: destination nodes (and their incident edges) are split across the
8 NeuronCores; the small weight matrices are replicated.  Per layer each core
computes K/V rows for ALL nodes (cheap matmuls) so the per-edge gathers stay
local, does the segment softmax + weighted aggregation for its own node shard
via one-hot selector matmuls on the PE, and the per-layer hidden state is
exchanged with an AllGather (feature-major fp16).
"""

import os
import sys
import types

import numpy as np

# ---------------------------------------------------------------------------
# NTFF profile hook (absent antenv.axon_hooks on this image) so trace=True
# works under axon.
if "antenv.axon_hooks" not in sys.modules:
    _m = types.ModuleType("antenv.axon_hooks")
    _m._hook = None

    def _set(h):
        _m._hook = h

    def _get():
        return _m._hook

    _m.set_axon_ntff_profile_hook = _set
    _m.get_axon_ntff_profile_hook = _get
    sys.modules["antenv.axon_hooks"] = _m
    try:
        import antenv

        antenv.axon_hooks = _m
    except Exception:
        pass
    try:
        from trn_agent_boot.trn_boot import _ntff_profile_via_ctypes

        _m._hook = _ntff_profile_via_ctypes("/opt/axon/libaxon_pjrt.so")
    except Exception:
        pass

import concourse.bass as bass
import concourse.mybir as mybir
import concourse.tile as tile
from concourse import bacc
from concourse import bass_utils
from concourse.masks import make_identity

F16 = mybir.dt.float16
BF16 = mybir.dt.bfloat16
F32 = mybir.dt.float32
I16 = mybir.dt.int16
AX = mybir.AxisListType
OP = mybir.AluOpType
ACTF = mybir.ActivationFunctionType

# problem constants (hardcoded per the harness contract)
N, E, IN, H, C, G, OUT = 50000, 800000, 128, 8, 32, 64, 64
HC = H * C  # 256
NLAYERS = 4
NC = 8
NLOC = N // NC          # 6250
W = 49                  # windows of 128 dst rows per core
NPAD = W * 128          # 6272 local rows (incl dummies)
NPAD_ALL = NC * NPAD    # 50176 kv rows
HALF = NPAD_ALL // 2    # 25088 (int16-addressable half)
SCALE = float(1.0 / np.sqrt(C))
MASK_NEG = -30000.0

_BUILD_CACHE = {}
LAST_RESULT = None


def _build(ewl, ewh, run_layers):
    """Build + compile the SPMD program.  ewl/ewh: padded lo/hi edge slots
    per window (multiples of 128)."""
    skip_b = bool(int(os.environ.get("KSKIP_B", "0")))
    skip_c = bool(int(os.environ.get("KSKIP_C", "0")))
    nwin = int(os.environ.get("KWIN", str(W)))
    kops = int(os.environ.get("KOPS", "7"))
    key = (ewl, ewh, run_layers, skip_b, skip_c, nwin, kops)
    if key in _BUILD_CACHE:
        return _BUILD_CACHE[key]

    S = (ewl + ewh) // 128          # kv slots per window
    SL, SH = ewl // 128, ewh // 128
    EW = ewl + ewh

    nc = bacc.Bacc("TRN2", target_bir_lowering=False, debug=False,
                   enable_asserts=False, num_devices=NC)

    # ---- external inputs (per-core content, same shapes) ----
    xT_full = nc.dram_tensor("xT_full", [128, NPAD_ALL], F16, kind="ExternalInput")
    xT_loc = nc.dram_tensor("xT_loc", [128, NPAD], F16, kind="ExternalInput")
    w_kv = nc.dram_tensor("w_kv", [128, NLAYERS, 2, 2 * HC], F16, kind="ExternalInput")
    w_qs = nc.dram_tensor("w_qs", [128, NLAYERS, 2, 2 * HC], F16, kind="ExternalInput")
    b_kv = nc.dram_tensor("b_kv", [1, NLAYERS, 2 * HC], F16, kind="ExternalInput")
    b_qs = nc.dram_tensor("b_qs", [1, NLAYERS, 2 * HC], F16, kind="ExternalInput")
    ones1 = nc.dram_tensor("ones1", [1, 128], F16, kind="ExternalInput")
    w_fc = nc.dram_tensor("w_fc", [128, 2, OUT], F16, kind="ExternalInput")
    b_fc = nc.dram_tensor("b_fc", [G, OUT], F32, kind="ExternalInput")
    iota_in = nc.dram_tensor("iota_in", [128, 128], F16, kind="ExternalInput")
    idx_lo = nc.dram_tensor("idx_lo", [128, W * (ewl // 16)], I16, kind="ExternalInput")
    idx_hi = nc.dram_tensor("idx_hi", [128, W * (ewh // 16)], I16, kind="ExternalInput")
    idx_q = nc.dram_tensor("idx_q", [128, W * (EW // 16)], I16, kind="ExternalInput")
    dstf = nc.dram_tensor("dstf", [128, W * S], F16, kind="ExternalInput")
    maskw = nc.dram_tensor("maskw", [128, W * S], F32, kind="ExternalInput")
    gsel = nc.dram_tensor("gsel", [128, W * G], F16, kind="ExternalInput")

    out_d = nc.dram_tensor("out", [G, OUT], F32, kind="ExternalOutput")
    debug = bool(int(os.environ.get("KDEBUG", "0")))
    if debug:
        dbg_h = nc.dram_tensor("dbg_h", [2, 128, NPAD], F16, kind="ExternalOutput")
        dbg_kv = nc.dram_tensor("dbg_kv", [1024, 2 * HC], F16, kind="ExternalOutput")
        dbg_qs = nc.dram_tensor("dbg_qs", [1024, 2 * HC], F16, kind="ExternalOutput")
        dbg_pool = nc.dram_tensor("dbg_pool", [2, 128, OUT], F32, kind="ExternalOutput")

    # ---- internal DRAM ----
    kv_full = nc.dram_tensor("kv_full", [NPAD_ALL, 2 * HC], F16, kind="Internal")
    qskip_full = nc.dram_tensor("qskip_full", [NPAD, 2 * HC], F16, kind="Internal")
    hT_shard = nc.dram_tensor("hT_shard", [2, 128, NPAD], F16, kind="Internal")
    hT_all = nc.dram_tensor("hT_all", [NC, 2, 128, NPAD], F16, kind="Internal",
                            addr_space="Shared")
    pool_part = nc.dram_tensor("pool_part", [2, 128, OUT], F32, kind="Internal")
    pool_sum = nc.dram_tensor("pool_sum", [2, 128, OUT], F32, kind="Internal",
                              addr_space="Shared")

    SLAB = 896          # 7 node-tiles per slab
    NSLAB = NPAD // SLAB  # 7

    with tile.TileContext(nc) as tc:
        with tc.tile_pool(name="const", bufs=1) as cp, \
             tc.tile_pool(name="slab", bufs=2) as slabp, \
             tc.tile_pool(name="kvb", bufs=2) as kvbp, \
             tc.tile_pool(name="win", bufs=2) as winp, \
             tc.tile_pool(name="psA", bufs=2, space="PSUM") as psA, \
             tc.tile_pool(name="psB", bufs=2, space="PSUM") as psB, \
             tc.tile_pool(name="psT", bufs=2, space="PSUM") as psT:

            # ---- load constants ----
            wkv_sb = cp.tile([128, NLAYERS, 2, 2 * HC], F16)
            wqs_sb = cp.tile([128, NLAYERS, 2, 2 * HC], F16)
            bkv_sb = cp.tile([1, NLAYERS, 2 * HC], F16)
            bqs_sb = cp.tile([1, NLAYERS, 2 * HC], F16)
            ones_sb = cp.tile([1, 128], F16)
            wfc_sb = cp.tile([128, 2, OUT], F16)
            bfc_sb = cp.tile([G, OUT], F32)
            iota_sb = cp.tile([128, 128], F16)
            idxlo_sb = cp.tile([128, W * (ewl // 16)], I16)
            idxhi_sb = cp.tile([128, W * (ewh // 16)], I16)
            idxq_sb = cp.tile([128, W * (EW // 16)], I16)
            dstf_sb = cp.tile([128, W * S], F16)
            maskw_sb = cp.tile([128, W * S], F32)
            gsel_sb = cp.tile([128, W * G], F16)
            ident_sb = cp.tile([128, 128], F16)
            pool_acc = cp.tile([128, 2, OUT], F32)

            for t, d in [(wkv_sb, w_kv), (wqs_sb, w_qs), (bkv_sb, b_kv),
                         (bqs_sb, b_qs), (ones_sb, ones1), (wfc_sb, w_fc),
                         (bfc_sb, b_fc), (iota_sb, iota_in), (idxlo_sb, idx_lo),
                         (idxhi_sb, idx_hi), (idxq_sb, idx_q), (dstf_sb, dstf),
                         (maskw_sb, maskw), (gsel_sb, gsel)]:
                nc.sync.dma_start(out=t[:], in_=d.ap())
            make_identity(nc, ident_sb[:])

            for layer in range(run_layers):
                KH = 1 if layer == 0 else 2

                # ===== Phase A: kv_full = [K|V] rows for all nodes =====
                for cc in range(NC):
                    for sl in range(NSLAB):
                        slabs = []
                        for kh in range(KH):
                            st = slabp.tile([128, SLAB], F16, tag="slab", bufs=4)
                            if layer == 0:
                                src_ap = xT_full.ap()[:, cc * NPAD + sl * SLAB:
                                                      cc * NPAD + (sl + 1) * SLAB]
                            else:
                                src_ap = hT_all.ap()[cc, kh, :,
                                                     sl * SLAB:(sl + 1) * SLAB]
                            nc.sync.dma_start(out=st[:], in_=src_ap)
                            slabs.append(st)
                        kvb = kvbp.tile([128, 7, 2 * HC], F16, tag="kvb")
                        for j in range(7):
                            ti = sl * 7 + j
                            ps = psA.tile([128, 2 * HC], F32, tag="psA")
                            for kh in range(KH):
                                nc.tensor.matmul(
                                    ps[:], lhsT=slabs[kh][:, j * 128:(j + 1) * 128],
                                    rhs=wkv_sb[:, layer, kh, :],
                                    start=(kh == 0), stop=False)
                            nc.tensor.matmul(
                                ps[:], lhsT=ones_sb[:],
                                rhs=bkv_sb[:, layer, :],
                                start=False, stop=True)
                            eng = nc.vector if (j % 2 == 0) else nc.scalar
                            if eng is nc.vector:
                                nc.vector.tensor_copy(kvb[:, j, :], ps[:])
                            else:
                                nc.scalar.activation(kvb[:, j, :], ps[:], ACTF.Copy)
                        dst_ap = kv_full.ap()[cc * NPAD + sl * SLAB:
                                              cc * NPAD + (sl + 1) * SLAB, :]
                        dst_ap = dst_ap.rearrange("(t p) e -> p t e", p=128)
                        nc.sync.dma_start(out=dst_ap, in_=kvb[:])

                # ===== Phase A2: q/skip rows for own shard =====
                for sl in range(NSLAB):
                    slabs = []
                    for kh in range(KH):
                        st = slabp.tile([128, SLAB], F16, tag="slab", bufs=4)
                        if layer == 0:
                            src_ap = xT_loc.ap()[:, sl * SLAB:(sl + 1) * SLAB]
                        else:
                            src_ap = hT_shard.ap()[kh, :, sl * SLAB:(sl + 1) * SLAB]
                        nc.sync.dma_start(out=st[:], in_=src_ap)
                        slabs.append(st)
                    qsb = kvbp.tile([128, 7, 2 * HC], F16, tag="kvb")
                    for j in range(7):
                        ps = psA.tile([128, 2 * HC], F32, tag="psA")
                        for kh in range(KH):
                            nc.tensor.matmul(
                                ps[:], lhsT=slabs[kh][:, j * 128:(j + 1) * 128],
                                rhs=wqs_sb[:, layer, kh, :],
                                start=(kh == 0), stop=False)
                        nc.tensor.matmul(ps[:], lhsT=ones_sb[:],
                                         rhs=bqs_sb[:, layer, :],
                                         start=False, stop=True)
                        if j % 2 == 0:
                            nc.vector.tensor_copy(qsb[:, j, :], ps[:])
                        else:
                            nc.scalar.activation(qsb[:, j, :], ps[:], ACTF.Copy)
                    dst_ap = qskip_full.ap()[sl * SLAB:(sl + 1) * SLAB, :]
                    dst_ap = dst_ap.rearrange("(t p) e -> p t e", p=128)
                    nc.sync.dma_start(out=dst_ap, in_=qsb[:])

                if debug and layer == 0:
                    nc.sync.dma_start(
                        out=dbg_kv.ap().rearrange("(t p) e -> p t e", p=128),
                        in_=kv_full.ap()[:1024, :].rearrange("(t p) e -> p t e", p=128))
                    nc.sync.dma_start(
                        out=dbg_qs.ap().rearrange("(t p) e -> p t e", p=128),
                        in_=qskip_full.ap()[:1024, :].rearrange("(t p) e -> p t e", p=128))

                # ===== Phase B: windows =====
                last = layer == NLAYERS - 1
                if skip_b:
                    continue
                if last:
                    pool_ps = [psT.tile([128, OUT], F32, tag=f"pps{kh}", bufs=1,
                                        name=f"pool_ps{kh}")
                               for kh in range(2)]

                def gather_chunked(out_tile, slot0, in_ap, idx_sb, col0, n,
                                   elem, estep=None):
                    done = 0
                    while done < n:
                        cur = min(1024, n - done)
                        nc.gpsimd.dma_gather(
                            out_ap=out_tile[:, slot0 + done // 128:
                                            slot0 + (done + cur) // 128, :],
                            in_ap=in_ap,
                            idxs_ap=idx_sb[:, col0 + done // 16:
                                           col0 + (done + cur) // 16],
                            num_idxs=cur, num_idxs_reg=cur, elem_size=elem,
                            elem_step=estep, single_packet=True)
                        done += cur

                for w in range(nwin):
                    kv_t = winp.tile([128, S, 2 * HC], F16, tag="kvt", bufs=2)
                    gather_chunked(kv_t, 0, kv_full.ap()[0:HALF, :],
                                   idxlo_sb, w * (ewl // 16), ewl, 2 * HC)
                    gather_chunked(kv_t, SL, kv_full.ap()[HALF:NPAD_ALL, :],
                                   idxhi_sb, w * (ewh // 16), ewh, 2 * HC)
                    q_t = winp.tile([128, S, HC], F16, tag="qt", bufs=2)
                    gather_chunked(q_t, 0, qskip_full.ap()[:, 0:HC],
                                   idxq_sb, w * (EW // 16), EW, HC,
                                   estep=2 * HC)
                    skip_w = winp.tile([128, HC], F16, tag="skw", bufs=2)
                    nc.sync.dma_start(
                        out=skip_w[:],
                        in_=qskip_full.ap()[w * 128:(w + 1) * 128, HC:2 * HC])
                    if kops < 2:
                        continue

                    # logits
                    qk = winp.tile([128, S, HC], F16, tag="qkpx", bufs=2)
                    nc.vector.tensor_tensor(qk[:], q_t[:], kv_t[:, :, 0:HC], OP.mult)
                    logits = winp.tile([128, S * H], F32, tag="lg", bufs=2)
                    nc.vector.tensor_reduce(
                        logits[:], qk[:].rearrange("p s (h c) -> p (s h) c", c=C),
                        axis=AX.X, op=OP.add)
                    ml = winp.tile([128, S * H], F32, tag="ml", bufs=2)
                    nc.vector.scalar_tensor_tensor(
                        out=ml[:].rearrange("p (s h) -> p s h", h=H),
                        in0=logits[:].rearrange("p (s h) -> p s h", h=H),
                        scalar=SCALE,
                        in1=maskw_sb[:, w * S:(w + 1) * S, None].to_broadcast(
                            (128, S, H)),
                        op0=OP.mult, op1=OP.add)
                    p_bf = winp.tile([128, S * H], BF16, tag="p", bufs=2)
                    nc.scalar.activation(p_bf[:], ml[:], ACTF.Exp)
                    if kops < 3:
                        continue

                    # wv (+ p columns)
                    pX = winp.tile([128, S, HC], BF16, tag="qkpx", bufs=2)
                    nc.scalar.activation(
                        pX[:].rearrange("p s (h c) -> p s h c", c=C),
                        p_bf[:].rearrange("p (s h) -> p s h", h=H)[:, :, :, None]
                        .to_broadcast((128, S, H, C)),
                        ACTF.Copy)
                    wv = winp.tile([128, S, HC + H], BF16, tag="wv", bufs=2)
                    nc.vector.tensor_tensor(
                        wv[:, :, 0:HC], kv_t[:, :, HC:2 * HC], pX[:], OP.mult)
                    nc.vector.tensor_copy(
                        wv[:, :, HC:HC + H],
                        p_bf[:].rearrange("p (s h) -> p s h", h=H))

                    if kops < 4:
                        continue
                    # one-hot selector
                    dstX = winp.tile([128, S, 128], F16, tag="dxsel", bufs=3)
                    nc.scalar.activation(
                        dstX[:],
                        dstf_sb[:, w * S:(w + 1) * S, None].to_broadcast(
                            (128, S, 128)),
                        ACTF.Copy)
                    sel = winp.tile([128, S, 128], BF16, tag="dxsel", bufs=3)
                    nc.vector.tensor_tensor(
                        sel[:], dstX[:],
                        iota_sb[:, None, :].to_broadcast((128, S, 128)),
                        OP.is_equal)

                    if kops < 5:
                        continue
                    # aggregate
                    agg = psB.tile([128, HC + H], F32, tag="agg")
                    for s in range(S):
                        nc.tensor.matmul(agg[:], lhsT=sel[:, s, :], rhs=wv[:, s, :],
                                         start=(s == 0), stop=(s == S - 1))

                    if kops < 6:
                        continue
                    # epilogue
                    rs0 = winp.tile([128, H], F32, tag="rs0", bufs=2)
                    nc.vector.tensor_scalar_add(rs0[:], agg[:, HC:HC + H], 1e-16)
                    rs = winp.tile([128, H], F32, tag="rs", bufs=2)
                    nc.vector.reciprocal(rs[:], rs0[:])
                    tmp = winp.tile([128, HC], F32, tag="tmp", bufs=2)
                    nc.vector.tensor_tensor(
                        tmp[:].rearrange("p (h c) -> p h c", c=C),
                        agg[:, 0:HC].rearrange("p (h c) -> p h c", c=C),
                        rs[:, :, None].to_broadcast((128, H, C)),
                        OP.mult)
                    tmp2 = winp.tile([128, HC], F32, tag="tmp2", bufs=2)
                    nc.vector.tensor_tensor(tmp2[:], tmp[:], skip_w[:], OP.add)
                    h_nm = winp.tile([128, HC], F16, tag="hnm", bufs=2)
                    nc.scalar.activation(h_nm[:], tmp2[:], ACTF.Relu)

                    if kops < 7:
                        continue
                    if last:
                        for kh in range(2):
                            nc.tensor.matmul(
                                pool_ps[kh][:],
                                lhsT=h_nm[:, kh * 128:(kh + 1) * 128],
                                rhs=gsel_sb[:, w * G:(w + 1) * G],
                                start=(w == 0), stop=(w == W - 1),
                                skip_group_check=True)
                    else:
                        hstage = winp.tile([128, 2, 128], F16, tag="hst", bufs=2)
                        for kh in range(2):
                            trp = psT.tile([128, 128], F16, tag="trp", bufs=1)
                            nc.tensor.transpose(
                                trp[:], h_nm[:, kh * 128:(kh + 1) * 128],
                                ident_sb[:])
                            if kh == 0:
                                nc.vector.tensor_copy(hstage[:, kh, :], trp[:])
                            else:
                                nc.scalar.activation(hstage[:, kh, :], trp[:],
                                                     ACTF.Copy)
                        nc.sync.dma_start(
                            out=hT_shard.ap().rearrange("k p n -> p k n")[
                                :, :, w * 128:(w + 1) * 128],
                            in_=hstage[:])

                # ===== Phase C =====
                if not last:
                    if not skip_c:
                        nc.gpsimd.collective_compute(
                            "AllGather", OP.bypass,
                            replica_groups=[list(range(NC))],
                            ins=[hT_shard.ap()], outs=[hT_all.ap()])
                    if debug:
                        nc.sync.dma_start(out=dbg_h.ap(), in_=hT_shard.ap())
                else:
                    for kh in range(2):
                        nc.vector.tensor_copy(pool_acc[:, kh, :], pool_ps[kh][:])
                    nc.sync.dma_start(
                        out=pool_part.ap().rearrange("k p o -> p k o"),
                        in_=pool_acc[:])
                    if not skip_c:
                        nc.gpsimd.collective_compute(
                            "AllReduce", OP.add,
                            replica_groups=[list(range(NC))],
                            ins=[pool_part.ap()], outs=[pool_sum.ap()])
                    pooled = cp.tile([128, 2, OUT], F32)
                    nc.sync.dma_start(
                        out=pooled[:],
                        in_=pool_sum.ap().rearrange("k p o -> p k o"))
                    if debug:
                        nc.sync.dma_start(out=dbg_pool.ap(), in_=pool_sum.ap())
                    pooled16 = cp.tile([128, 2, OUT], F16)
                    nc.vector.tensor_copy(pooled16[:], pooled[:])
                    fin = psB.tile([G, OUT], F32, tag="fin", bufs=1)
                    for kh in range(2):
                        nc.tensor.matmul(fin[:], lhsT=pooled16[:, kh, :],
                                         rhs=wfc_sb[:, kh, :],
                                         start=(kh == 0), stop=(kh == 1))
                    out_sb = cp.tile([G, OUT], F32)
                    nc.vector.tensor_tensor(out_sb[:], fin[:], bfc_sb[:], OP.add)
                    nc.sync.dma_start(out=out_d.ap(), in_=out_sb[:])

            if run_layers < NLAYERS:
                # partial build (debug): emit output anyway so run works
                out_sb2 = cp.tile([G, OUT], F32)
                nc.vector.memset(out_sb2[:], 0.0)
                nc.sync.dma_start(out=out_d.ap(), in_=out_sb2[:])

    nc.compile()
    _BUILD_CACHE[key] = nc
    return nc


def _wrap16(a):
    """[W, n] int array -> [128, W*(n//16)] int16 gather-index layout."""
    Wn, n = a.shape
    out = a.reshape(Wn, n // 16, 16).transpose(2, 0, 1).reshape(16, Wn * (n // 16))
    return np.tile(out, (8, 1)).astype(np.int16)


def _pos128(a, dtype):
    """[W, EW] per-position array -> [128, W*S] SBUF layout."""
    Wn, n = a.shape
    return np.ascontiguousarray(
        a.reshape(Wn, n // 128, 128).transpose(2, 0, 1).reshape(
            128, Wn * (n // 128)).astype(dtype))


def _host_prep(inputs):
    x = np.asarray(inputs["x"], np.float32)
    ei = np.asarray(inputs["edge_index"]).astype(np.int64)
    batch = np.asarray(inputs["batch"]).astype(np.int64)
    src, dst = ei[0], ei[1]

    f16 = np.float16
    # -- weights (shared across cores) --
    def pack_w(W0a, W0b, Wla, Wlb):
        # -> [128, NLAYERS, 2, 512] f16 ; layer0 kh=1 is zero
        w = np.zeros((128, NLAYERS, 2, 2 * HC), f16)
        w[:, 0, 0, 0:HC] = np.asarray(W0a, np.float32).astype(f16)
        w[:, 0, 0, HC:] = np.asarray(W0b, np.float32).astype(f16)
        for l in range(NLAYERS - 1):
            a = np.asarray(Wla[l], np.float32).astype(f16)
            b = np.asarray(Wlb[l], np.float32).astype(f16)
            for kh in range(2):
                w[:, l + 1, kh, 0:HC] = a[kh * 128:(kh + 1) * 128]
                w[:, l + 1, kh, HC:] = b[kh * 128:(kh + 1) * 128]
        return w

    wkv = pack_w(inputs["Wk0"], inputs["Wv0"], inputs["Wk"], inputs["Wv"])
    wqs = pack_w(inputs["Wq0"], inputs["Ws0"], inputs["Wq"], inputs["Ws"])

    def pack_b(b0a, b0b, bla, blb):
        b = np.zeros((1, NLAYERS, 2 * HC), f16)
        b[0, 0, 0:HC] = np.asarray(b0a, np.float32).astype(f16)
        b[0, 0, HC:] = np.asarray(b0b, np.float32).astype(f16)
        for l in range(NLAYERS - 1):
            b[0, l + 1, 0:HC] = np.asarray(bla[l], np.float32).astype(f16)
            b[0, l + 1, HC:] = np.asarray(blb[l], np.float32).astype(f16)
        return b

    bkv = pack_b(inputs["bk0"], inputs["bv0"], inputs["bk"], inputs["bv"])
    bqs = pack_b(inputs["bq0"], inputs["bs0"], inputs["bq"], inputs["bs"])

    wfc = np.asarray(inputs["Wfc"], np.float32).astype(f16)
    wfc_p = np.ascontiguousarray(
        wfc.reshape(2, 128, OUT).transpose(1, 0, 2))
    bfc_rep = np.tile(np.asarray(inputs["bfc"], np.float32)[None, :], (G, 1))

    iota = np.tile(np.arange(128, dtype=f16)[None, :], (128, 1))
    ones1 = np.ones((1, 128), f16)

    # -- x transposed, padded layout --
    xT = np.zeros((128, NPAD_ALL), f16)
    xt = np.ascontiguousarray(x.T.astype(f16))
    for cc in range(NC):
        xT[:, cc * NPAD:cc * NPAD + NLOC] = xt[:, cc * NLOC:(cc + 1) * NLOC]

    counts = np.bincount(batch, minlength=G).astype(np.float32)
    inv_counts = 1.0 / np.maximum(counts, 1.0)

    # -- per-core edge structures --
    core_of = dst // NLOC
    row_of_src = (src // NLOC) * NPAD + (src % NLOC)

    # global padded sizes
    ewl_max = ewh_max = 0
    percore = []
    for c in range(NC):
        m = core_of == c
        s_row = row_of_src[m]
        dloc = dst[m] - c * NLOC
        w_of = dloc // 128
        is_lo = s_row < HALF
        order = np.lexsort((dloc, ~is_lo, w_of))
        s_row, dloc, w_of, is_lo = (s_row[order], dloc[order],
                                    w_of[order], is_lo[order])
        nlo = np.bincount(w_of[is_lo], minlength=W)
        nhi = np.bincount(w_of[~is_lo], minlength=W)
        ewl_max = max(ewl_max, int(nlo.max()))
        ewh_max = max(ewh_max, int(nhi.max()))
        percore.append((s_row, dloc, w_of, is_lo, nlo, nhi))

    ewl = -(-ewl_max // 128) * 128
    ewh = -(-ewh_max // 128) * 128
    EW = ewl + ewh
    S = EW // 128

    in_maps = []
    shared = dict(w_kv=wkv, w_qs=wqs, b_kv=bkv, b_qs=bqs, ones1=ones1,
                  w_fc=wfc_p, b_fc=bfc_rep.astype(np.float32),
                  iota_in=iota, xT_full=xT)
    for c in range(NC):
        s_row, dloc, w_of, is_lo, nlo, nhi = percore[c]
        ilo = np.zeros((W, ewl), np.int64)
        ihi = np.zeros((W, ewh), np.int64)
        iq = np.zeros((W, EW), np.int64)
        dpos = np.zeros((W, EW), np.int64)
        mpos = np.full((W, EW), MASK_NEG, np.float64)
        # window start offsets in the sorted arrays
        wstart = np.searchsorted(w_of, np.arange(W))
        wend = np.searchsorted(w_of, np.arange(W) + 1)
        for w in range(W):
            a, b = int(wstart[w]), int(wend[w])
            k = int(nlo[w])
            # lo edges [a, a+k), hi edges [a+k, b)
            ilo[w, :k] = s_row[a:a + k]
            iq[w, :k] = dloc[a:a + k]
            dpos[w, :k] = dloc[a:a + k] % 128
            mpos[w, :k] = 0.0
            nh = b - (a + k)
            ihi[w, :nh] = s_row[a + k:b] - HALF
            iq[w, ewl:ewl + nh] = dloc[a + k:b]
            dpos[w, ewl:ewl + nh] = dloc[a + k:b] % 128
            mpos[w, ewl:ewl + nh] = 0.0

        gs = np.zeros((W, 128, G), f16)
        nodes = np.arange(NPAD)
        real = nodes < NLOC
        gn = batch[c * NLOC + nodes[real]]
        gs_flat = np.zeros((NPAD, G), np.float32)
        gs_flat[nodes[real], gn] = inv_counts[gn]
        gs = gs_flat.reshape(W, 128, G)
        gsel_in = np.ascontiguousarray(
            gs.transpose(1, 0, 2).reshape(128, W * G).astype(f16))

        in_maps.append(dict(
            shared,
            xT_loc=np.ascontiguousarray(xT[:, c * NPAD:(c + 1) * NPAD]),
            idx_lo=_wrap16(ilo), idx_hi=_wrap16(ihi), idx_q=_wrap16(iq),
            dstf=_pos128(dpos, f16), maskw=_pos128(mpos, np.float32),
            gsel=gsel_in))
    return in_maps, ewl, ewh


def kernel(**inputs):
    global LAST_RESULT
    in_maps, ewl, ewh = _host_prep(inputs)
    run_layers = int(os.environ.get("RUN_LAYERS", str(NLAYERS)))
    nc = _build(ewl, ewh, run_layers)
    trace = bool(int(os.environ.get("KTRACE", "0")))
    res = bass_utils.run_bass_kernel_spmd(
        nc, in_maps, core_ids=list(range(NC)), trace=trace)
    LAST_RESULT = res
    return res.results[0]["out"].astype(np.float32)


# revision 20
# speedup vs baseline: 1.3514x; 1.3514x over previous
"""Trainium2 Bass kernel for nn_EnhancedGNNTransformerEncoder (4-layer
TransformerConv GNN + mean-pool + linear head).

Sharding: destination nodes (and their incident edges) are split across the
8 NeuronCores; the small weight matrices are replicated.  Per layer each core
computes K/V rows for ALL nodes (cheap matmuls) so the per-edge gathers stay
local, does the segment softmax + weighted aggregation for its own node shard
via one-hot selector matmuls on the PE, and the per-layer hidden state is
exchanged with an AllGather (feature-major fp16).
"""

import os
import sys
import types

import numpy as np

# ---------------------------------------------------------------------------
# NTFF profile hook (absent antenv.axon_hooks on this image) so trace=True
# works under axon.
if "antenv.axon_hooks" not in sys.modules:
    _m = types.ModuleType("antenv.axon_hooks")
    _m._hook = None

    def _set(h):
        _m._hook = h

    def _get():
        return _m._hook

    _m.set_axon_ntff_profile_hook = _set
    _m.get_axon_ntff_profile_hook = _get
    sys.modules["antenv.axon_hooks"] = _m
    try:
        import antenv

        antenv.axon_hooks = _m
    except Exception:
        pass
    try:
        from trn_agent_boot.trn_boot import _ntff_profile_via_ctypes

        _m._hook = _ntff_profile_via_ctypes("/opt/axon/libaxon_pjrt.so")
    except Exception:
        pass

import concourse.bass as bass
import concourse.mybir as mybir
import concourse.tile as tile
from concourse import bacc
from concourse import bass_utils
from concourse.masks import make_identity

F16 = mybir.dt.float16
BF16 = mybir.dt.bfloat16
F32 = mybir.dt.float32
I16 = mybir.dt.int16
AX = mybir.AxisListType
OP = mybir.AluOpType
ACTF = mybir.ActivationFunctionType

# problem constants (hardcoded per the harness contract)
N, E, IN, H, C, G, OUT = 50000, 800000, 128, 8, 32, 64, 64
HC = H * C  # 256
NLAYERS = 4
NC = 8
NLOC = N // NC          # 6250
W = 49                  # windows of 128 dst rows per core
NPAD = W * 128          # 6272 local rows (incl dummies)
NPAD_ALL = NC * NPAD    # 50176 kv rows
HALF = NPAD_ALL // 2    # 25088 (int16-addressable half)
SCALE = float(1.0 / np.sqrt(C))
MASK_NEG = -30000.0

_BUILD_CACHE = {}
LAST_RESULT = None


def _build(ewl, ewh, run_layers):
    """Build + compile the SPMD program.  ewl/ewh: padded lo/hi edge slots
    per window (multiples of 128)."""
    skip_b = bool(int(os.environ.get("KSKIP_B", "0")))
    skip_c = bool(int(os.environ.get("KSKIP_C", "0")))
    nwin = int(os.environ.get("KWIN", str(W)))
    kops = int(os.environ.get("KOPS", "7"))
    key = (ewl, ewh, run_layers, skip_b, skip_c, nwin, kops)
    if key in _BUILD_CACHE:
        return _BUILD_CACHE[key]

    S = (ewl + ewh) // 128          # kv slots per window
    SL, SH = ewl // 128, ewh // 128
    EW = ewl + ewh

    nc = bacc.Bacc("TRN2", target_bir_lowering=False, debug=False,
                   enable_asserts=False, num_devices=NC,
                   num_swdge_queues=4)

    # ---- external inputs (per-core content, same shapes) ----
    xT_full = nc.dram_tensor("xT_full", [128, NPAD_ALL], F16, kind="ExternalInput")
    xT_loc = nc.dram_tensor("xT_loc", [128, NPAD], F16, kind="ExternalInput")
    w_kv = nc.dram_tensor("w_kv", [128, NLAYERS, 2, 2 * HC], F16, kind="ExternalInput")
    w_qs = nc.dram_tensor("w_qs", [128, NLAYERS, 2, 2 * HC], F16, kind="ExternalInput")
    b_kv = nc.dram_tensor("b_kv", [1, NLAYERS, 2 * HC], F16, kind="ExternalInput")
    b_qs = nc.dram_tensor("b_qs", [1, NLAYERS, 2 * HC], F16, kind="ExternalInput")
    ones1 = nc.dram_tensor("ones1", [1, 128], F16, kind="ExternalInput")
    w_fc = nc.dram_tensor("w_fc", [128, 2, OUT], F16, kind="ExternalInput")
    b_fc = nc.dram_tensor("b_fc", [G, OUT], F32, kind="ExternalInput")
    iota_in = nc.dram_tensor("iota_in", [128, 128], F16, kind="ExternalInput")
    idx_lo = nc.dram_tensor("idx_lo", [128, W * (ewl // 16)], I16, kind="ExternalInput")
    idx_hi = nc.dram_tensor("idx_hi", [128, W * (ewh // 16)], I16, kind="ExternalInput")
    dstf = nc.dram_tensor("dstf", [128, W * S], F16, kind="ExternalInput")
    dstT = nc.dram_tensor("dstT", [128, W * EW], F16, kind="ExternalInput")
    iotaP = nc.dram_tensor("iotaP", [128, EW], F16, kind="ExternalInput")
    maskw = nc.dram_tensor("maskw", [128, W * S], F32, kind="ExternalInput")
    gsel = nc.dram_tensor("gsel", [128, W * G], F16, kind="ExternalInput")

    out_d = nc.dram_tensor("out", [G, OUT], F32, kind="ExternalOutput")
    debug = bool(int(os.environ.get("KDEBUG", "0")))
    if debug:
        dbg_h = nc.dram_tensor("dbg_h", [2, 128, NPAD], F16, kind="ExternalOutput")
        dbg_kv = nc.dram_tensor("dbg_kv", [1024, 2 * HC], F16, kind="ExternalOutput")
        dbg_qs = nc.dram_tensor("dbg_qs", [1024, 2 * HC], F16, kind="ExternalOutput")
        dbg_pool = nc.dram_tensor("dbg_pool", [2, 128, OUT], F32, kind="ExternalOutput")

    # ---- internal DRAM ----
    kv_full = nc.dram_tensor("kv_full", [NPAD_ALL, 2 * HC], F16, kind="Internal")
    qskip_full = nc.dram_tensor("qskip_full", [NPAD, 2 * HC], F16, kind="Internal")
    hT_shard = nc.dram_tensor("hT_shard", [2, 128, NPAD], F16, kind="Internal")
    hT_all = nc.dram_tensor("hT_all", [NC, 2, 128, NPAD], F16, kind="Internal",
                            addr_space="Shared")
    pool_part = nc.dram_tensor("pool_part", [2, 128, OUT], F32, kind="Internal")
    pool_sum = nc.dram_tensor("pool_sum", [2, 128, OUT], F32, kind="Internal",
                              addr_space="Shared")

    SLAB = 896          # 7 node-tiles per slab
    NSLAB = NPAD // SLAB  # 7

    with tile.TileContext(nc) as tc:
        with tc.tile_pool(name="const", bufs=1) as cp, \
             tc.tile_pool(name="slab", bufs=2) as slabp, \
             tc.tile_pool(name="kvb", bufs=2) as kvbp, \
             tc.tile_pool(name="win", bufs=2) as winp, \
             tc.tile_pool(name="psA", bufs=2, space="PSUM") as psA, \
             tc.tile_pool(name="psB", bufs=2, space="PSUM") as psB, \
             tc.tile_pool(name="psT", bufs=2, space="PSUM") as psT, \
             tc.tile_pool(name="psQ", bufs=2, space="PSUM") as psQ:

            # ---- load constants ----
            wkv_sb = cp.tile([128, NLAYERS, 2, 2 * HC], F16)
            wqs_sb = cp.tile([128, NLAYERS, 2, 2 * HC], F16)
            bkv_sb = cp.tile([1, NLAYERS, 2 * HC], F16)
            bqs_sb = cp.tile([1, NLAYERS, 2 * HC], F16)
            ones_sb = cp.tile([1, 128], F16)
            wfc_sb = cp.tile([128, 2, OUT], F16)
            bfc_sb = cp.tile([G, OUT], F32)
            iota_sb = cp.tile([128, 128], F16)
            idxlo_sb = cp.tile([128, W * (ewl // 16)], I16)
            idxhi_sb = cp.tile([128, W * (ewh // 16)], I16)
            dstf_sb = cp.tile([128, W * S], F16)
            iotaP_sb = cp.tile([128, EW], F16)
            maskw_sb = cp.tile([128, W * S], F32)
            gsel_sb = cp.tile([128, W * G], F16)
            ident_sb = cp.tile([128, 128], F16)
            pool_acc = cp.tile([128, 2, OUT], F32)

            for t, d in [(wkv_sb, w_kv), (wqs_sb, w_qs), (bkv_sb, b_kv),
                         (bqs_sb, b_qs), (ones_sb, ones1), (wfc_sb, w_fc),
                         (bfc_sb, b_fc), (iota_sb, iota_in), (idxlo_sb, idx_lo),
                         (idxhi_sb, idx_hi), (dstf_sb, dstf),
                         (maskw_sb, maskw), (gsel_sb, gsel),
                         (iotaP_sb, iotaP)]:
                nc.sync.dma_start(out=t[:], in_=d.ap())
            make_identity(nc, ident_sb[:])

            for layer in range(run_layers):
                KH = 1 if layer == 0 else 2

                # ===== Phase A: kv_full = [K|V] rows for all nodes =====
                for cc in range(NC):
                    for sl in range(NSLAB):
                        slabs = []
                        for kh in range(KH):
                            st = slabp.tile([128, SLAB], F16, tag="slab", bufs=4)
                            if layer == 0:
                                src_ap = xT_full.ap()[:, cc * NPAD + sl * SLAB:
                                                      cc * NPAD + (sl + 1) * SLAB]
                            else:
                                src_ap = hT_all.ap()[cc, kh, :,
                                                     sl * SLAB:(sl + 1) * SLAB]
                            nc.sync.dma_start(out=st[:], in_=src_ap)
                            slabs.append(st)
                        kvb = kvbp.tile([128, 7, 2 * HC], F16, tag="kvb")
                        for j in range(7):
                            ti = sl * 7 + j
                            ps = psA.tile([128, 2 * HC], F32, tag="psA")
                            for kh in range(KH):
                                nc.tensor.matmul(
                                    ps[:], lhsT=slabs[kh][:, j * 128:(j + 1) * 128],
                                    rhs=wkv_sb[:, layer, kh, :],
                                    start=(kh == 0), stop=False)
                            nc.tensor.matmul(
                                ps[:], lhsT=ones_sb[:],
                                rhs=bkv_sb[:, layer, :],
                                start=False, stop=True)
                            eng = nc.vector if (j % 2 == 0) else nc.scalar
                            if eng is nc.vector:
                                nc.vector.tensor_copy(kvb[:, j, :], ps[:])
                            else:
                                nc.scalar.activation(kvb[:, j, :], ps[:], ACTF.Copy)
                        dst_ap = kv_full.ap()[cc * NPAD + sl * SLAB:
                                              cc * NPAD + (sl + 1) * SLAB, :]
                        dst_ap = dst_ap.rearrange("(t p) e -> p t e", p=128)
                        nc.sync.dma_start(out=dst_ap, in_=kvb[:])

                # ===== Phase A2: q/skip rows for own shard =====
                for sl in range(NSLAB):
                    slabs = []
                    for kh in range(KH):
                        st = slabp.tile([128, SLAB], F16, tag="slab", bufs=4)
                        if layer == 0:
                            src_ap = xT_loc.ap()[:, sl * SLAB:(sl + 1) * SLAB]
                        else:
                            src_ap = hT_shard.ap()[kh, :, sl * SLAB:(sl + 1) * SLAB]
                        nc.sync.dma_start(out=st[:], in_=src_ap)
                        slabs.append(st)
                    qsb = kvbp.tile([128, 7, 2 * HC], F16, tag="kvb")
                    for j in range(7):
                        ps = psA.tile([128, 2 * HC], F32, tag="psA")
                        for kh in range(KH):
                            nc.tensor.matmul(
                                ps[:], lhsT=slabs[kh][:, j * 128:(j + 1) * 128],
                                rhs=wqs_sb[:, layer, kh, :],
                                start=(kh == 0), stop=False)
                        nc.tensor.matmul(ps[:], lhsT=ones_sb[:],
                                         rhs=bqs_sb[:, layer, :],
                                         start=False, stop=True)
                        if j % 2 == 0:
                            nc.vector.tensor_copy(qsb[:, j, :], ps[:])
                        else:
                            nc.scalar.activation(qsb[:, j, :], ps[:], ACTF.Copy)
                    dst_ap = qskip_full.ap()[sl * SLAB:(sl + 1) * SLAB, :]
                    dst_ap = dst_ap.rearrange("(t p) e -> p t e", p=128)
                    nc.sync.dma_start(out=dst_ap, in_=qsb[:])

                if debug and layer == 0:
                    nc.sync.dma_start(
                        out=dbg_kv.ap().rearrange("(t p) e -> p t e", p=128),
                        in_=kv_full.ap()[:1024, :].rearrange("(t p) e -> p t e", p=128))
                    nc.sync.dma_start(
                        out=dbg_qs.ap().rearrange("(t p) e -> p t e", p=128),
                        in_=qskip_full.ap()[:1024, :].rearrange("(t p) e -> p t e", p=128))

                # ===== Phase B: windows =====
                last = layer == NLAYERS - 1
                if skip_b:
                    continue
                if last:
                    nc.vector.memset(pool_acc[:], 0.0)

                qctr = [0]

                def gather_chunked(out_tile, slot0, in_ap, idx_sb, col0, n,
                                   elem, estep=None):
                    done = 0
                    while done < n:
                        cur = min(1024, n - done)
                        nc.gpsimd.dma_gather(
                            out_ap=out_tile[:, slot0 + done // 128:
                                            slot0 + (done + cur) // 128, :],
                            in_ap=in_ap,
                            idxs_ap=idx_sb[:, col0 + done // 16:
                                           col0 + (done + cur) // 16],
                            num_idxs=cur, num_idxs_reg=cur, elem_size=elem,
                            elem_step=estep, single_packet=True,
                            queue_num=qctr[0] % 4)
                        qctr[0] += 1
                        done += cur

                for w in range(nwin):
                    kv_t = winp.tile([128, S, 2 * HC], F16, tag="kvt", bufs=2)
                    gather_chunked(kv_t, 0, kv_full.ap()[0:HALF, :],
                                   idxlo_sb, w * (ewl // 16), ewl, 2 * HC)
                    gather_chunked(kv_t, SL, kv_full.ap()[HALF:NPAD_ALL, :],
                                   idxhi_sb, w * (ewh // 16), ewh, 2 * HC)
                    qs_w = winp.tile([128, 2 * HC], F16, tag="qsw", bufs=2)
                    nc.sync.dma_start(
                        out=qs_w[:],
                        in_=qskip_full.ap()[w * 128:(w + 1) * 128, :])
                    skip_w = qs_w[:, HC:2 * HC]
                    dstT_w = winp.tile([128, EW], F16, tag="dstT", bufs=2)
                    nc.sync.dma_start(
                        out=dstT_w[:], in_=dstT.ap()[:, w * EW:(w + 1) * EW])
                    selT = winp.tile([128, S, 128], F16, tag="dxsel", bufs=4)
                    nc.vector.tensor_tensor(
                        selT[:].rearrange("p s e -> p (s e)"),
                        dstT_w[:], iotaP_sb[:], OP.is_equal)
                    q_t = winp.tile([128, S, HC], F16, tag="qt", bufs=2)
                    for sq in range(S):
                        qe = psQ.tile([128, HC], F32, tag="qe")
                        nc.tensor.matmul(qe[:], lhsT=selT[:, sq, :],
                                         rhs=qs_w[:, 0:HC],
                                         start=True, stop=True,
                                         skip_group_check=True)
                        nc.scalar.activation(q_t[:, sq, :], qe[:], ACTF.Copy)
                    if kops < 2:
                        continue

                    # logits
                    qk = winp.tile([128, S, HC], F16, tag="qkpx", bufs=2)
                    nc.vector.tensor_tensor(qk[:], q_t[:], kv_t[:, :, 0:HC], OP.mult)
                    logits = winp.tile([128, S * H], F32, tag="lg", bufs=2)
                    nc.vector.tensor_reduce(
                        logits[:], qk[:].rearrange("p s (h c) -> p (s h) c", c=C),
                        axis=AX.X, op=OP.add)
                    ml = winp.tile([128, S * H], F32, tag="ml", bufs=2)
                    nc.vector.scalar_tensor_tensor(
                        out=ml[:].rearrange("p (s h) -> p s h", h=H),
                        in0=logits[:].rearrange("p (s h) -> p s h", h=H),
                        scalar=SCALE,
                        in1=maskw_sb[:, w * S:(w + 1) * S, None].to_broadcast(
                            (128, S, H)),
                        op0=OP.mult, op1=OP.add)
                    p_bf = winp.tile([128, S * H], BF16, tag="p", bufs=2)
                    nc.scalar.activation(p_bf[:], ml[:], ACTF.Exp)
                    if kops < 3:
                        continue

                    # wv (+ p columns)
                    pX = winp.tile([128, S, HC], BF16, tag="qkpx", bufs=2)
                    nc.scalar.activation(
                        pX[:].rearrange("p s (h c) -> p s h c", c=C),
                        p_bf[:].rearrange("p (s h) -> p s h", h=H)[:, :, :, None]
                        .to_broadcast((128, S, H, C)),
                        ACTF.Copy)
                    wv = winp.tile([128, S, HC + H], BF16, tag="wv", bufs=2)
                    nc.vector.tensor_tensor(
                        wv[:, :, 0:HC], kv_t[:, :, HC:2 * HC], pX[:], OP.mult)
                    nc.vector.tensor_copy(
                        wv[:, :, HC:HC + H],
                        p_bf[:].rearrange("p (s h) -> p s h", h=H))

                    if kops < 4:
                        continue
                    # one-hot selector
                    dstX = winp.tile([128, S, 128], F16, tag="dxsel", bufs=4)
                    nc.scalar.activation(
                        dstX[:],
                        dstf_sb[:, w * S:(w + 1) * S, None].to_broadcast(
                            (128, S, 128)),
                        ACTF.Copy)
                    sel = winp.tile([128, S, 128], BF16, tag="dxsel", bufs=4)
                    nc.vector.tensor_tensor(
                        sel[:], dstX[:],
                        iota_sb[:, None, :].to_broadcast((128, S, 128)),
                        OP.is_equal)

                    if kops < 5:
                        continue
                    # aggregate
                    agg = psB.tile([128, HC + H], F32, tag="agg")
                    for s in range(S):
                        nc.tensor.matmul(agg[:], lhsT=sel[:, s, :], rhs=wv[:, s, :],
                                         start=(s == 0), stop=(s == S - 1),
                                         skip_group_check=True)

                    if kops < 6:
                        continue
                    # epilogue
                    rs0 = winp.tile([128, H], F32, tag="rs0", bufs=2)
                    nc.vector.tensor_scalar_add(rs0[:], agg[:, HC:HC + H], 1e-16)
                    rs = winp.tile([128, H], F32, tag="rs", bufs=2)
                    nc.vector.reciprocal(rs[:], rs0[:])
                    tmp = winp.tile([128, HC], F32, tag="tmp", bufs=2)
                    nc.vector.tensor_tensor(
                        tmp[:].rearrange("p (h c) -> p h c", c=C),
                        agg[:, 0:HC].rearrange("p (h c) -> p h c", c=C),
                        rs[:, :, None].to_broadcast((128, H, C)),
                        OP.mult)
                    tmp2 = winp.tile([128, HC], F32, tag="tmp2", bufs=2)
                    nc.vector.tensor_tensor(tmp2[:], tmp[:], skip_w, OP.add)
                    h_nm = winp.tile([128, HC], F16, tag="hnm", bufs=2)
                    nc.scalar.activation(h_nm[:], tmp2[:], ACTF.Relu)

                    if kops < 7:
                        continue
                    if last:
                        for kh in range(2):
                            ptmp = psT.tile([128, OUT], F32, tag="trp",
                                            name="ptmp")
                            nc.tensor.matmul(
                                ptmp[:],
                                lhsT=h_nm[:, kh * 128:(kh + 1) * 128],
                                rhs=gsel_sb[:, w * G:(w + 1) * G],
                                start=True, stop=True,
                                skip_group_check=True)
                            nc.vector.tensor_tensor(
                                pool_acc[:, kh, :], pool_acc[:, kh, :],
                                ptmp[:], OP.add)
                    else:
                        hstage = winp.tile([128, 2, 128], F16, tag="hst", bufs=2)
                        for kh in range(2):
                            trp = psT.tile([128, 128], F16, tag="trp")
                            nc.tensor.transpose(
                                trp[:], h_nm[:, kh * 128:(kh + 1) * 128],
                                ident_sb[:])
                            if kh == 0:
                                nc.vector.tensor_copy(hstage[:, kh, :], trp[:])
                            else:
                                nc.scalar.activation(hstage[:, kh, :], trp[:],
                                                     ACTF.Copy)
                        nc.sync.dma_start(
                            out=hT_shard.ap().rearrange("k p n -> p k n")[
                                :, :, w * 128:(w + 1) * 128],
                            in_=hstage[:])

                # ===== Phase C =====
                if not last:
                    if not skip_c:
                        nc.gpsimd.collective_compute(
                            "AllGather", OP.bypass,
                            replica_groups=[list(range(NC))],
                            ins=[hT_shard.ap()], outs=[hT_all.ap()])
                    if debug:
                        nc.sync.dma_start(out=dbg_h.ap(), in_=hT_shard.ap())
                else:
                    nc.sync.dma_start(
                        out=pool_part.ap().rearrange("k p o -> p k o"),
                        in_=pool_acc[:])
                    if not skip_c:
                        nc.gpsimd.collective_compute(
                            "AllReduce", OP.add,
                            replica_groups=[list(range(NC))],
                            ins=[pool_part.ap()], outs=[pool_sum.ap()])
                    pooled = cp.tile([128, 2, OUT], F32)
                    nc.sync.dma_start(
                        out=pooled[:],
                        in_=pool_sum.ap().rearrange("k p o -> p k o"))
                    if debug:
                        nc.sync.dma_start(out=dbg_pool.ap(), in_=pool_sum.ap())
                    pooled16 = cp.tile([128, 2, OUT], F16)
                    nc.vector.tensor_copy(pooled16[:], pooled[:])
                    fin = psB.tile([G, OUT], F32, tag="agg")
                    for kh in range(2):
                        nc.tensor.matmul(fin[:], lhsT=pooled16[:, kh, :],
                                         rhs=wfc_sb[:, kh, :],
                                         start=(kh == 0), stop=(kh == 1))
                    out_sb = cp.tile([G, OUT], F32)
                    nc.vector.tensor_tensor(out_sb[:], fin[:], bfc_sb[:], OP.add)
                    nc.sync.dma_start(out=out_d.ap(), in_=out_sb[:])

            if run_layers < NLAYERS:
                # partial build (debug): emit output anyway so run works
                out_sb2 = cp.tile([G, OUT], F32)
                nc.vector.memset(out_sb2[:], 0.0)
                nc.sync.dma_start(out=out_d.ap(), in_=out_sb2[:])

    nc.compile()
    _BUILD_CACHE[key] = nc
    return nc


def _wrap16(a):
    """[W, n] int array -> [128, W*(n//16)] int16 gather-index layout."""
    Wn, n = a.shape
    out = a.reshape(Wn, n // 16, 16).transpose(2, 0, 1).reshape(16, Wn * (n // 16))
    return np.tile(out, (8, 1)).astype(np.int16)


def _pos128(a, dtype):
    """[W, EW] per-position array -> [128, W*S] SBUF layout."""
    Wn, n = a.shape
    return np.ascontiguousarray(
        a.reshape(Wn, n // 128, 128).transpose(2, 0, 1).reshape(
            128, Wn * (n // 128)).astype(dtype))


def _host_prep(inputs):
    x = np.asarray(inputs["x"], np.float32)
    ei = np.asarray(inputs["edge_index"]).astype(np.int64)
    batch = np.asarray(inputs["batch"]).astype(np.int64)
    src, dst = ei[0], ei[1]

    f16 = np.float16
    # -- weights (shared across cores) --
    def pack_w(W0a, W0b, Wla, Wlb):
        # -> [128, NLAYERS, 2, 512] f16 ; layer0 kh=1 is zero
        w = np.zeros((128, NLAYERS, 2, 2 * HC), f16)
        w[:, 0, 0, 0:HC] = np.asarray(W0a, np.float32).astype(f16)
        w[:, 0, 0, HC:] = np.asarray(W0b, np.float32).astype(f16)
        for l in range(NLAYERS - 1):
            a = np.asarray(Wla[l], np.float32).astype(f16)
            b = np.asarray(Wlb[l], np.float32).astype(f16)
            for kh in range(2):
                w[:, l + 1, kh, 0:HC] = a[kh * 128:(kh + 1) * 128]
                w[:, l + 1, kh, HC:] = b[kh * 128:(kh + 1) * 128]
        return w

    wkv = pack_w(inputs["Wk0"], inputs["Wv0"], inputs["Wk"], inputs["Wv"])
    wqs = pack_w(inputs["Wq0"], inputs["Ws0"], inputs["Wq"], inputs["Ws"])

    def pack_b(b0a, b0b, bla, blb):
        b = np.zeros((1, NLAYERS, 2 * HC), f16)
        b[0, 0, 0:HC] = np.asarray(b0a, np.float32).astype(f16)
        b[0, 0, HC:] = np.asarray(b0b, np.float32).astype(f16)
        for l in range(NLAYERS - 1):
            b[0, l + 1, 0:HC] = np.asarray(bla[l], np.float32).astype(f16)
            b[0, l + 1, HC:] = np.asarray(blb[l], np.float32).astype(f16)
        return b

    bkv = pack_b(inputs["bk0"], inputs["bv0"], inputs["bk"], inputs["bv"])
    bqs = pack_b(inputs["bq0"], inputs["bs0"], inputs["bq"], inputs["bs"])

    wfc = np.asarray(inputs["Wfc"], np.float32).astype(f16)
    wfc_p = np.ascontiguousarray(
        wfc.reshape(2, 128, OUT).transpose(1, 0, 2))
    bfc_rep = np.tile(np.asarray(inputs["bfc"], np.float32)[None, :], (G, 1))

    iota = np.tile(np.arange(128, dtype=f16)[None, :], (128, 1))
    ones1 = np.ones((1, 128), f16)

    # -- x transposed, padded layout --
    xT = np.zeros((128, NPAD_ALL), f16)
    xt = np.ascontiguousarray(x.T.astype(f16))
    for cc in range(NC):
        xT[:, cc * NPAD:cc * NPAD + NLOC] = xt[:, cc * NLOC:(cc + 1) * NLOC]

    counts = np.bincount(batch, minlength=G).astype(np.float32)
    inv_counts = 1.0 / np.maximum(counts, 1.0)

    # -- per-core edge structures --
    core_of = dst // NLOC
    row_of_src = (src // NLOC) * NPAD + (src % NLOC)

    # global padded sizes
    ewl_max = ewh_max = 0
    percore = []
    for c in range(NC):
        m = core_of == c
        s_row = row_of_src[m]
        dloc = dst[m] - c * NLOC
        w_of = dloc // 128
        is_lo = s_row < HALF
        order = np.lexsort((dloc, ~is_lo, w_of))
        s_row, dloc, w_of, is_lo = (s_row[order], dloc[order],
                                    w_of[order], is_lo[order])
        nlo = np.bincount(w_of[is_lo], minlength=W)
        nhi = np.bincount(w_of[~is_lo], minlength=W)
        ewl_max = max(ewl_max, int(nlo.max()))
        ewh_max = max(ewh_max, int(nhi.max()))
        percore.append((s_row, dloc, w_of, is_lo, nlo, nhi))

    ewl = -(-ewl_max // 128) * 128
    ewh = -(-ewh_max // 128) * 128
    EW = ewl + ewh
    S = EW // 128

    in_maps = []
    shared = dict(w_kv=wkv, w_qs=wqs, b_kv=bkv, b_qs=bqs, ones1=ones1,
                  w_fc=wfc_p, b_fc=bfc_rep.astype(np.float32),
                  iota_in=iota, xT_full=xT,
                  iotaP=np.tile(np.arange(128, dtype=f16)[:, None], (1, EW)))
    for c in range(NC):
        s_row, dloc, w_of, is_lo, nlo, nhi = percore[c]
        ilo = np.zeros((W, ewl), np.int64)
        ihi = np.zeros((W, ewh), np.int64)
        iq = np.zeros((W, EW), np.int64)
        dpos = np.zeros((W, EW), np.int64)
        mpos = np.full((W, EW), MASK_NEG, np.float64)
        # window start offsets in the sorted arrays
        wstart = np.searchsorted(w_of, np.arange(W))
        wend = np.searchsorted(w_of, np.arange(W) + 1)
        for w in range(W):
            a, b = int(wstart[w]), int(wend[w])
            k = int(nlo[w])
            # lo edges [a, a+k), hi edges [a+k, b)
            ilo[w, :k] = s_row[a:a + k]
            iq[w, :k] = dloc[a:a + k]
            dpos[w, :k] = dloc[a:a + k] % 128
            mpos[w, :k] = 0.0
            nh = b - (a + k)
            ihi[w, :nh] = s_row[a + k:b] - HALF
            iq[w, ewl:ewl + nh] = dloc[a + k:b]
            dpos[w, ewl:ewl + nh] = dloc[a + k:b] % 128
            mpos[w, ewl:ewl + nh] = 0.0

        gs = np.zeros((W, 128, G), f16)
        nodes = np.arange(NPAD)
        real = nodes < NLOC
        gn = batch[c * NLOC + nodes[real]]
        gs_flat = np.zeros((NPAD, G), np.float32)
        gs_flat[nodes[real], gn] = inv_counts[gn]
        gs = gs_flat.reshape(W, 128, G)
        gsel_in = np.ascontiguousarray(
            gs.transpose(1, 0, 2).reshape(128, W * G).astype(f16))

        in_maps.append(dict(
            shared,
            xT_loc=np.ascontiguousarray(xT[:, c * NPAD:(c + 1) * NPAD]),
            idx_lo=_wrap16(ilo), idx_hi=_wrap16(ihi),
            dstf=_pos128(dpos, f16), maskw=_pos128(mpos, np.float32),
            dstT=np.tile(dpos.reshape(1, W * EW), (128, 1)).astype(f16),
            gsel=gsel_in))
    return in_maps, ewl, ewh


def kernel(**inputs):
    global LAST_RESULT
    in_maps, ewl, ewh = _host_prep(inputs)
    run_layers = int(os.environ.get("RUN_LAYERS", str(NLAYERS)))
    nc = _build(ewl, ewh, run_layers)
    trace = bool(int(os.environ.get("KTRACE", "0")))
    res = bass_utils.run_bass_kernel_spmd(
        nc, in_maps, core_ids=list(range(NC)), trace=trace)
    LAST_RESULT = res
    return res.results[0]["out"].astype(np.float32)


# revision 24
# speedup vs baseline: 1.5890x; 1.1758x over previous
"""Trainium2 Bass kernel for nn_EnhancedGNNTransformerEncoder (4-layer
TransformerConv GNN + mean-pool + linear head).

Sharding: destination nodes (and their incident edges) are split across the
8 NeuronCores; the small weight matrices are replicated.  Per layer each core
computes K/V rows for ALL nodes (cheap matmuls) so the per-edge gathers stay
local, does the segment softmax + weighted aggregation for its own node shard
via one-hot selector matmuls on the PE, and the per-layer hidden state is
exchanged with an AllGather (feature-major fp16).
"""

import os
import sys
import types

import numpy as np

# ---------------------------------------------------------------------------
# NTFF profile hook (absent antenv.axon_hooks on this image) so trace=True
# works under axon.
if "antenv.axon_hooks" not in sys.modules:
    _m = types.ModuleType("antenv.axon_hooks")
    _m._hook = None

    def _set(h):
        _m._hook = h

    def _get():
        return _m._hook

    _m.set_axon_ntff_profile_hook = _set
    _m.get_axon_ntff_profile_hook = _get
    sys.modules["antenv.axon_hooks"] = _m
    try:
        import antenv

        antenv.axon_hooks = _m
    except Exception:
        pass
    try:
        from trn_agent_boot.trn_boot import _ntff_profile_via_ctypes

        _m._hook = _ntff_profile_via_ctypes("/opt/axon/libaxon_pjrt.so")
    except Exception:
        pass

import concourse.bass as bass
import concourse.mybir as mybir
import concourse.tile as tile
from concourse import bacc
from concourse import bass_utils
from concourse.masks import make_identity

F16 = mybir.dt.float16
BF16 = mybir.dt.bfloat16
F32 = mybir.dt.float32
I16 = mybir.dt.int16
AX = mybir.AxisListType
OP = mybir.AluOpType
ACTF = mybir.ActivationFunctionType

# problem constants (hardcoded per the harness contract)
N, E, IN, H, C, G, OUT = 50000, 800000, 128, 8, 32, 64, 64
HC = H * C  # 256
NLAYERS = 4
NC = 8
NLOC = N // NC          # 6250
W = 50                  # windows of 128 dst rows per core
NPAD = W * 128          # 6272 local rows (incl dummies)
NPAD_ALL = NC * NPAD    # 50176 kv rows
HALF = NPAD_ALL // 2    # 25088 (int16-addressable half)
SCALE = float(1.0 / np.sqrt(C))
MASK_NEG = -30000.0

_BUILD_CACHE = {}
LAST_RESULT = None


def _build(ewl, ewh, run_layers, use_bias=True):
    """Build + compile the SPMD program.  ewl/ewh: padded lo/hi edge slots
    per window (multiples of 128)."""
    skip_b = bool(int(os.environ.get("KSKIP_B", "0")))
    skip_c = bool(int(os.environ.get("KSKIP_C", "0")))
    nwin = int(os.environ.get("KWIN", str(W)))
    kops = int(os.environ.get("KOPS", "7"))
    key = (ewl, ewh, run_layers, skip_b, skip_c, nwin, kops, use_bias)
    if key in _BUILD_CACHE:
        return _BUILD_CACHE[key]

    S = (ewl + ewh) // 128          # kv slots per window
    SL, SH = ewl // 128, ewh // 128
    EW = ewl + ewh

    nc = bacc.Bacc("TRN2", target_bir_lowering=False, debug=False,
                   enable_asserts=False, num_devices=NC,
                   num_swdge_queues=4)

    # ---- external inputs (per-core content, same shapes) ----
    xT_full = nc.dram_tensor("xT_full", [128, NPAD_ALL], F16, kind="ExternalInput")
    xT_loc = nc.dram_tensor("xT_loc", [128, NPAD], F16, kind="ExternalInput")
    w_kv = nc.dram_tensor("w_kv", [128, NLAYERS, 2, 2 * HC], F16, kind="ExternalInput")
    w_qs = nc.dram_tensor("w_qs", [128, NLAYERS, 2, 2 * HC], F16, kind="ExternalInput")
    b_kv = nc.dram_tensor("b_kv", [1, NLAYERS, 2 * HC], F16, kind="ExternalInput")
    b_qs = nc.dram_tensor("b_qs", [1, NLAYERS, 2 * HC], F16, kind="ExternalInput")
    ones1 = nc.dram_tensor("ones1", [1, 128], F16, kind="ExternalInput")
    w_fc = nc.dram_tensor("w_fc", [128, 2, OUT], F16, kind="ExternalInput")
    b_fc = nc.dram_tensor("b_fc", [G, OUT], F32, kind="ExternalInput")
    iota_in = nc.dram_tensor("iota_in", [128, 128], F16, kind="ExternalInput")
    idx_lo = nc.dram_tensor("idx_lo", [128, W * (ewl // 16)], I16, kind="ExternalInput")
    idx_hi = nc.dram_tensor("idx_hi", [128, W * (ewh // 16)], I16, kind="ExternalInput")
    dstf = nc.dram_tensor("dstf", [128, W * S], F16, kind="ExternalInput")
    dstT = nc.dram_tensor("dstT", [128, W * EW], F16, kind="ExternalInput")
    iotaP = nc.dram_tensor("iotaP", [128, EW], F16, kind="ExternalInput")
    maskw = nc.dram_tensor("maskw", [128, W * S], F32, kind="ExternalInput")
    gsel = nc.dram_tensor("gsel", [128, W * G], F16, kind="ExternalInput")

    out_d = nc.dram_tensor("out", [G, OUT], F32, kind="ExternalOutput")
    debug = bool(int(os.environ.get("KDEBUG", "0")))
    if debug:
        dbg_h = nc.dram_tensor("dbg_h", [2, 128, NPAD], F16, kind="ExternalOutput")
        dbg_kv = nc.dram_tensor("dbg_kv", [1024, 2 * HC], F16, kind="ExternalOutput")
        dbg_qs = nc.dram_tensor("dbg_qs", [1024, 2 * HC], F16, kind="ExternalOutput")
        dbg_pool = nc.dram_tensor("dbg_pool", [2, 128, OUT], F32, kind="ExternalOutput")

    # ---- internal DRAM ----
    kv_full = nc.dram_tensor("kv_full", [NPAD_ALL, 2 * HC], F16, kind="Internal")
    qskip_full = nc.dram_tensor("qskip_full", [NPAD, 2 * HC], F16, kind="Internal")
    hT_shard = nc.dram_tensor("hT_shard", [2, 128, NPAD], F16, kind="Internal")
    hT_all = nc.dram_tensor("hT_all", [NC, 2, 128, NPAD], F16, kind="Internal",
                            addr_space="Shared")
    pool_part = nc.dram_tensor("pool_part", [2, 128, OUT], F32, kind="Internal")
    pool_sum = nc.dram_tensor("pool_sum", [2, 128, OUT], F32, kind="Internal",
                              addr_space="Shared")

    SLAB = 1280         # 10 node-tiles per slab
    NSLAB = NPAD // SLAB  # 5

    with tile.TileContext(nc) as tc:
        with tc.tile_pool(name="const", bufs=1) as cp, \
             tc.tile_pool(name="slab", bufs=2) as slabp, \
             tc.tile_pool(name="kvb", bufs=2) as kvbp, \
             tc.tile_pool(name="win", bufs=2) as winp, \
             tc.tile_pool(name="psA", bufs=2, space="PSUM") as psA, \
             tc.tile_pool(name="psB", bufs=2, space="PSUM") as psB, \
             tc.tile_pool(name="psT", bufs=2, space="PSUM") as psT, \
             tc.tile_pool(name="psQ", bufs=2, space="PSUM") as psQ:

            # ---- load constants ----
            wkv_sb = cp.tile([128, NLAYERS, 2, 2 * HC], F16)
            wqs_sb = cp.tile([128, NLAYERS, 2, 2 * HC], F16)
            bkv_sb = cp.tile([1, NLAYERS, 2 * HC], F16)
            bqs_sb = cp.tile([1, NLAYERS, 2 * HC], F16)
            ones_sb = cp.tile([1, 128], F16)
            wfc_sb = cp.tile([128, 2, OUT], F16)
            bfc_sb = cp.tile([G, OUT], F32)
            iota_sb = cp.tile([128, 128], F16)
            idxlo_sb = cp.tile([128, W * (ewl // 16)], I16)
            idxhi_sb = cp.tile([128, W * (ewh // 16)], I16)
            dstf_sb = cp.tile([128, W * S], F16)
            iotaP_sb = cp.tile([128, EW], F16)
            maskw_sb = cp.tile([128, W * S], F32)
            gsel_sb = cp.tile([128, W * G], F16)
            ident_sb = cp.tile([128, 128], F16)
            pool_acc = cp.tile([128, 2, OUT], F32)

            for t, d in [(wkv_sb, w_kv), (wqs_sb, w_qs), (bkv_sb, b_kv),
                         (bqs_sb, b_qs), (ones_sb, ones1), (wfc_sb, w_fc),
                         (bfc_sb, b_fc), (iota_sb, iota_in), (idxlo_sb, idx_lo),
                         (idxhi_sb, idx_hi), (dstf_sb, dstf),
                         (maskw_sb, maskw), (gsel_sb, gsel),
                         (iotaP_sb, iotaP)]:
                nc.sync.dma_start(out=t[:], in_=d.ap())
            make_identity(nc, ident_sb[:])

            for layer in range(run_layers):
                KH = 1 if layer == 0 else 2

                # ===== Phase A: kv_full = [K|V] rows for all nodes =====
                for cc in range(NC):
                    for sl in range(NSLAB):
                        slabs = []
                        for kh in range(KH):
                            st = slabp.tile([128, SLAB], F16, tag="slab", bufs=4)
                            if layer == 0:
                                src_ap = xT_full.ap()[:, cc * NPAD + sl * SLAB:
                                                      cc * NPAD + (sl + 1) * SLAB]
                            else:
                                src_ap = hT_all.ap()[cc, kh, :,
                                                     sl * SLAB:(sl + 1) * SLAB]
                            nc.sync.dma_start(out=st[:], in_=src_ap)
                            slabs.append(st)
                        for half5 in range(2):
                          kvb = kvbp.tile([128, 5, 2 * HC], F16, tag="kvb")
                          for j5 in range(5):
                            j = half5 * 5 + j5
                            ti = sl * 10 + j
                            ps = psA.tile([128, 2 * HC], F32, tag="psA")
                            for kh in range(KH):
                                nc.tensor.matmul(
                                    ps[:], lhsT=slabs[kh][:, j * 128:(j + 1) * 128],
                                    rhs=wkv_sb[:, layer, kh, :],
                                    start=(kh == 0),
                                    stop=(not use_bias and kh == KH - 1))
                            if use_bias:
                                nc.tensor.matmul(
                                    ps[:], lhsT=ones_sb[:],
                                    rhs=bkv_sb[:, layer, :],
                                    start=False, stop=True)
                            if j % 2 == 0:
                                nc.vector.tensor_copy(kvb[:, j5, :], ps[:])
                            else:
                                nc.scalar.activation(kvb[:, j5, :], ps[:], ACTF.Copy)
                          base = cc * NPAD + sl * SLAB + half5 * 640
                          dst_ap = kv_full.ap()[base:base + 640, :]
                          dst_ap = dst_ap.rearrange("(t p) e -> p t e", p=128)
                          nc.sync.dma_start(out=dst_ap, in_=kvb[:])

                # ===== Phase A2: q/skip rows for own shard =====
                for sl in range(NSLAB):
                    slabs = []
                    for kh in range(KH):
                        st = slabp.tile([128, SLAB], F16, tag="slab", bufs=4)
                        if layer == 0:
                            src_ap = xT_loc.ap()[:, sl * SLAB:(sl + 1) * SLAB]
                        else:
                            src_ap = hT_shard.ap()[kh, :, sl * SLAB:(sl + 1) * SLAB]
                        nc.sync.dma_start(out=st[:], in_=src_ap)
                        slabs.append(st)
                    for half5 in range(2):
                        qsb = kvbp.tile([128, 5, 2 * HC], F16, tag="kvb")
                        for j5 in range(5):
                            j = half5 * 5 + j5
                            ps = psA.tile([128, 2 * HC], F32, tag="psA")
                            for kh in range(KH):
                                nc.tensor.matmul(
                                    ps[:], lhsT=slabs[kh][:, j * 128:(j + 1) * 128],
                                    rhs=wqs_sb[:, layer, kh, :],
                                    start=(kh == 0),
                                    stop=(not use_bias and kh == KH - 1))
                            if use_bias:
                                nc.tensor.matmul(ps[:], lhsT=ones_sb[:],
                                                 rhs=bqs_sb[:, layer, :],
                                                 start=False, stop=True)
                            if j % 2 == 0:
                                nc.vector.tensor_copy(qsb[:, j5, :], ps[:])
                            else:
                                nc.scalar.activation(qsb[:, j5, :], ps[:],
                                                     ACTF.Copy)
                        qbase = sl * SLAB + half5 * 640
                        dst_ap = qskip_full.ap()[qbase:qbase + 640, :]
                        dst_ap = dst_ap.rearrange("(t p) e -> p t e", p=128)
                        nc.sync.dma_start(out=dst_ap, in_=qsb[:])

                if debug and layer == 0:
                    nc.sync.dma_start(
                        out=dbg_kv.ap().rearrange("(t p) e -> p t e", p=128),
                        in_=kv_full.ap()[:1024, :].rearrange("(t p) e -> p t e", p=128))
                    nc.sync.dma_start(
                        out=dbg_qs.ap().rearrange("(t p) e -> p t e", p=128),
                        in_=qskip_full.ap()[:1024, :].rearrange("(t p) e -> p t e", p=128))

                # ===== Phase B: windows =====
                last = layer == NLAYERS - 1
                if skip_b:
                    continue
                if last:
                    nc.vector.memset(pool_acc[:], 0.0)

                qctr = [0]

                def gather_chunked(out_tile, slot0, in_ap, idx_sb, col0, n,
                                   elem, estep=None):
                    done = 0
                    while done < n:
                        cur = min(1024, n - done)
                        nc.gpsimd.dma_gather(
                            out_ap=out_tile[:, slot0 + done // 128:
                                            slot0 + (done + cur) // 128, :],
                            in_ap=in_ap,
                            idxs_ap=idx_sb[:, col0 + done // 16:
                                           col0 + (done + cur) // 16],
                            num_idxs=cur, num_idxs_reg=cur, elem_size=elem,
                            elem_step=estep, single_packet=True,
                            queue_num=qctr[0] % 4)
                        qctr[0] += 1
                        done += cur

                for w in range(nwin):
                    kv_t = winp.tile([128, S, 2 * HC], F16, tag="kvt", bufs=3)
                    gather_chunked(kv_t, 0, kv_full.ap()[0:HALF, :],
                                   idxlo_sb, w * (ewl // 16), ewl, 2 * HC)
                    gather_chunked(kv_t, SL, kv_full.ap()[HALF:NPAD_ALL, :],
                                   idxhi_sb, w * (ewh // 16), ewh, 2 * HC)
                    qs_w = winp.tile([128, 2 * HC], F16, tag="qsw", bufs=2)
                    nc.sync.dma_start(
                        out=qs_w[:],
                        in_=qskip_full.ap()[w * 128:(w + 1) * 128, :])
                    skip_w = qs_w[:, HC:2 * HC]
                    dstT_w = winp.tile([128, EW], F16, tag="dstT", bufs=2)
                    nc.sync.dma_start(
                        out=dstT_w[:], in_=dstT.ap()[:, w * EW:(w + 1) * EW])
                    selT = winp.tile([128, S, 128], F16, tag="dxsel", bufs=4)
                    nc.vector.tensor_tensor(
                        selT[:].rearrange("p s e -> p (s e)"),
                        dstT_w[:], iotaP_sb[:], OP.is_equal)
                    q_t = winp.tile([128, S, HC], F16, tag="qt", bufs=2)
                    for sq in range(S):
                        qe = psQ.tile([128, HC], F32, tag="qe")
                        nc.tensor.matmul(qe[:], lhsT=selT[:, sq, :],
                                         rhs=qs_w[:, 0:HC],
                                         start=True, stop=True,
                                         skip_group_check=True)
                        nc.scalar.activation(q_t[:, sq, :], qe[:], ACTF.Copy)
                    if kops < 2:
                        continue

                    # logits
                    qk = winp.tile([128, S, HC], F16, tag="qkpx", bufs=2)
                    nc.vector.tensor_tensor(qk[:], q_t[:], kv_t[:, :, 0:HC], OP.mult)
                    logits = winp.tile([128, S * H], F32, tag="lg", bufs=2)
                    nc.vector.tensor_reduce(
                        logits[:], qk[:].rearrange("p s (h c) -> p (s h) c", c=C),
                        axis=AX.X, op=OP.add)
                    ml = winp.tile([128, S * H], F32, tag="ml", bufs=2)
                    nc.vector.scalar_tensor_tensor(
                        out=ml[:].rearrange("p (s h) -> p s h", h=H),
                        in0=logits[:].rearrange("p (s h) -> p s h", h=H),
                        scalar=SCALE,
                        in1=maskw_sb[:, w * S:(w + 1) * S, None].to_broadcast(
                            (128, S, H)),
                        op0=OP.mult, op1=OP.add)
                    p_bf = winp.tile([128, S * H], BF16, tag="p", bufs=2)
                    nc.scalar.activation(p_bf[:], ml[:], ACTF.Exp)
                    if kops < 3:
                        continue

                    # wv (+ p columns)
                    pX = winp.tile([128, S, HC], BF16, tag="qkpx", bufs=2)
                    nc.scalar.activation(
                        pX[:].rearrange("p s (h c) -> p s h c", c=C),
                        p_bf[:].rearrange("p (s h) -> p s h", h=H)[:, :, :, None]
                        .to_broadcast((128, S, H, C)),
                        ACTF.Copy)
                    wv = winp.tile([128, S, HC + H], BF16, tag="wv", bufs=2)
                    nc.vector.tensor_tensor(
                        wv[:, :, 0:HC], kv_t[:, :, HC:2 * HC], pX[:], OP.mult)
                    nc.vector.tensor_copy(
                        wv[:, :, HC:HC + H],
                        p_bf[:].rearrange("p (s h) -> p s h", h=H))

                    if kops < 4:
                        continue
                    # one-hot selector
                    dstX = winp.tile([128, S, 128], F16, tag="dxsel", bufs=4)
                    nc.scalar.activation(
                        dstX[:],
                        dstf_sb[:, w * S:(w + 1) * S, None].to_broadcast(
                            (128, S, 128)),
                        ACTF.Copy)
                    sel = winp.tile([128, S, 128], BF16, tag="dxsel", bufs=4)
                    nc.vector.tensor_tensor(
                        sel[:], dstX[:],
                        iota_sb[:, None, :].to_broadcast((128, S, 128)),
                        OP.is_equal)

                    if kops < 5:
                        continue
                    # aggregate
                    agg = psB.tile([128, HC + H], F32, tag="agg")
                    for s in range(S):
                        nc.tensor.matmul(agg[:], lhsT=sel[:, s, :], rhs=wv[:, s, :],
                                         start=(s == 0), stop=(s == S - 1),
                                         skip_group_check=True)

                    if kops < 6:
                        continue
                    # epilogue
                    rs0 = winp.tile([128, H], F32, tag="rs0", bufs=2)
                    nc.vector.tensor_scalar_add(rs0[:], agg[:, HC:HC + H], 1e-16)
                    rs = winp.tile([128, H], F32, tag="rs", bufs=2)
                    nc.vector.reciprocal(rs[:], rs0[:])
                    tmp = winp.tile([128, HC], F32, tag="tmp", bufs=2)
                    nc.vector.tensor_tensor(
                        tmp[:].rearrange("p (h c) -> p h c", c=C),
                        agg[:, 0:HC].rearrange("p (h c) -> p h c", c=C),
                        rs[:, :, None].to_broadcast((128, H, C)),
                        OP.mult)
                    tmp2 = winp.tile([128, HC], F32, tag="tmp2", bufs=2)
                    nc.vector.tensor_tensor(tmp2[:], tmp[:], skip_w, OP.add)
                    h_nm = winp.tile([128, HC], F16, tag="hnm", bufs=2)
                    nc.scalar.activation(h_nm[:], tmp2[:], ACTF.Relu)

                    if kops < 7:
                        continue
                    if last:
                        for kh in range(2):
                            ptmp = psT.tile([128, OUT], F32, tag="trp",
                                            name="ptmp")
                            nc.tensor.matmul(
                                ptmp[:],
                                lhsT=h_nm[:, kh * 128:(kh + 1) * 128],
                                rhs=gsel_sb[:, w * G:(w + 1) * G],
                                start=True, stop=True,
                                skip_group_check=True)
                            nc.vector.tensor_tensor(
                                pool_acc[:, kh, :], pool_acc[:, kh, :],
                                ptmp[:], OP.add)
                    else:
                        hstage = winp.tile([128, 2, 128], F16, tag="hst", bufs=2)
                        for kh in range(2):
                            trp = psT.tile([128, 128], F16, tag="trp")
                            nc.tensor.transpose(
                                trp[:], h_nm[:, kh * 128:(kh + 1) * 128],
                                ident_sb[:])
                            if kh == 0:
                                nc.vector.tensor_copy(hstage[:, kh, :], trp[:])
                            else:
                                nc.scalar.activation(hstage[:, kh, :], trp[:],
                                                     ACTF.Copy)
                        nc.sync.dma_start(
                            out=hT_shard.ap().rearrange("k p n -> p k n")[
                                :, :, w * 128:(w + 1) * 128],
                            in_=hstage[:])

                # ===== Phase C =====
                if not last:
                    if not skip_c:
                        nc.gpsimd.collective_compute(
                            "AllGather", OP.bypass,
                            replica_groups=[list(range(NC))],
                            ins=[hT_shard.ap()], outs=[hT_all.ap()])
                    if debug:
                        nc.sync.dma_start(out=dbg_h.ap(), in_=hT_shard.ap())
                else:
                    nc.sync.dma_start(
                        out=pool_part.ap().rearrange("k p o -> p k o"),
                        in_=pool_acc[:])
                    if not skip_c:
                        nc.gpsimd.collective_compute(
                            "AllReduce", OP.add,
                            replica_groups=[list(range(NC))],
                            ins=[pool_part.ap()], outs=[pool_sum.ap()])
                    pooled = cp.tile([128, 2, OUT], F32)
                    nc.sync.dma_start(
                        out=pooled[:],
                        in_=pool_sum.ap().rearrange("k p o -> p k o"))
                    if debug:
                        nc.sync.dma_start(out=dbg_pool.ap(), in_=pool_sum.ap())
                    pooled16 = cp.tile([128, 2, OUT], F16)
                    nc.vector.tensor_copy(pooled16[:], pooled[:])
                    fin = psB.tile([G, OUT], F32, tag="agg")
                    for kh in range(2):
                        nc.tensor.matmul(fin[:], lhsT=pooled16[:, kh, :],
                                         rhs=wfc_sb[:, kh, :],
                                         start=(kh == 0), stop=(kh == 1))
                    out_sb = cp.tile([G, OUT], F32)
                    nc.vector.tensor_tensor(out_sb[:], fin[:], bfc_sb[:], OP.add)
                    nc.sync.dma_start(out=out_d.ap(), in_=out_sb[:])

            if run_layers < NLAYERS:
                # partial build (debug): emit output anyway so run works
                out_sb2 = cp.tile([G, OUT], F32)
                nc.vector.memset(out_sb2[:], 0.0)
                nc.sync.dma_start(out=out_d.ap(), in_=out_sb2[:])

    nc.compile()
    _BUILD_CACHE[key] = nc
    return nc


def _wrap16(a):
    """[W, n] int array -> [128, W*(n//16)] int16 gather-index layout."""
    Wn, n = a.shape
    out = a.reshape(Wn, n // 16, 16).transpose(2, 0, 1).reshape(16, Wn * (n // 16))
    return np.tile(out, (8, 1)).astype(np.int16)


def _pos128(a, dtype):
    """[W, EW] per-position array -> [128, W*S] SBUF layout."""
    Wn, n = a.shape
    return np.ascontiguousarray(
        a.reshape(Wn, n // 128, 128).transpose(2, 0, 1).reshape(
            128, Wn * (n // 128)).astype(dtype))


def _host_prep(inputs):
    x = np.asarray(inputs["x"], np.float32)
    ei = np.asarray(inputs["edge_index"]).astype(np.int64)
    batch = np.asarray(inputs["batch"]).astype(np.int64)
    src, dst = ei[0], ei[1]

    f16 = np.float16
    def pack_w(W0a, W0b, Wla, Wlb):
        w = np.zeros((128, NLAYERS, 2, 2 * HC), f16)
        w[:, 0, 0, 0:HC] = np.asarray(W0a, np.float32).astype(f16)
        w[:, 0, 0, HC:] = np.asarray(W0b, np.float32).astype(f16)
        for l in range(NLAYERS - 1):
            a = np.asarray(Wla[l], np.float32).astype(f16)
            b = np.asarray(Wlb[l], np.float32).astype(f16)
            for kh in range(2):
                w[:, l + 1, kh, 0:HC] = a[kh * 128:(kh + 1) * 128]
                w[:, l + 1, kh, HC:] = b[kh * 128:(kh + 1) * 128]
        return w

    wkv = pack_w(inputs["Wk0"], inputs["Wv0"], inputs["Wk"], inputs["Wv"])
    wqs = pack_w(inputs["Wq0"], inputs["Ws0"], inputs["Wq"], inputs["Ws"])

    def pack_b(b0a, b0b, bla, blb):
        b = np.zeros((1, NLAYERS, 2 * HC), f16)
        b[0, 0, 0:HC] = np.asarray(b0a, np.float32).astype(f16)
        b[0, 0, HC:] = np.asarray(b0b, np.float32).astype(f16)
        for l in range(NLAYERS - 1):
            b[0, l + 1, 0:HC] = np.asarray(bla[l], np.float32).astype(f16)
            b[0, l + 1, HC:] = np.asarray(blb[l], np.float32).astype(f16)
        return b

    bkv = pack_b(inputs["bk0"], inputs["bv0"], inputs["bk"], inputs["bv"])
    bqs = pack_b(inputs["bq0"], inputs["bs0"], inputs["bq"], inputs["bs"])
    use_bias = bool(np.abs(bkv).max() > 0 or np.abs(bqs).max() > 0)

    wfc = np.asarray(inputs["Wfc"], np.float32).astype(f16)
    wfc_p = np.ascontiguousarray(wfc.reshape(2, 128, OUT).transpose(1, 0, 2))
    bfc_rep = np.tile(np.asarray(inputs["bfc"], np.float32)[None, :], (G, 1))

    iota = np.tile(np.arange(128, dtype=f16)[None, :], (128, 1))
    ones1 = np.ones((1, 128), f16)

    counts = np.bincount(batch, minlength=G).astype(np.float32)
    inv_counts = (1.0 / np.maximum(counts, 1.0)).astype(np.float32)

    # ---- balanced node->window assignment (per core) ----
    HALF_NODE = HALF  # row threshold
    perms = []        # per core: original local idx -> padded local row
    core_edges = []
    for c in range(NC):
        m = (dst >= c * NLOC) & (dst < (c + 1) * NLOC)
        s_c = src[m]
        dloc = dst[m] - c * NLOC
        # provisional src row (needs perms of src cores; fill later)
        core_edges.append((s_c, dloc))
        lo_e = ((s_c // NLOC) * NPAD + (s_c % NLOC)) < HALF_NODE
        lod = np.bincount(dloc[lo_e], minlength=NLOC).astype(np.int64)
        hid = np.bincount(dloc[~lo_e], minlength=NLOC).astype(np.int64)
        order = np.argsort(-(lod + hid), kind="stable")
        wlo = np.zeros(W); whi = np.zeros(W); wcnt = np.zeros(W, np.int64)
        wof = np.empty(NLOC, np.int64)
        for n in order:
            cand = np.where(wcnt < 128)[0]
            score = np.maximum(wlo[cand] + lod[n], whi[cand] + hid[n])
            j = cand[int(np.argmin(score))]
            wof[n] = j
            wlo[j] += lod[n]; whi[j] += hid[n]; wcnt[j] += 1
        # rank within window
        perm = np.empty(NLOC, np.int64)
        fill = np.zeros(W, np.int64)
        for n in range(NLOC):
            wn = wof[n]
            perm[n] = wn * 128 + fill[wn]
            fill[wn] += 1
        perms.append(perm)

    # src global row with permutation of the owner core
    def row_of(nodes):
        c_of = nodes // NLOC
        r = np.empty(len(nodes), np.int64)
        for c in range(NC):
            mm = c_of == c
            r[mm] = c * NPAD + perms[c][nodes[mm] % NLOC]
        return r

    # -- x transposed, padded + permuted layout --
    xT = np.zeros((128, NPAD_ALL), f16)
    xt = np.ascontiguousarray(x.T.astype(f16))
    for c in range(NC):
        cols = c * NPAD + perms[c]
        xT[:, cols] = xt[:, c * NLOC:(c + 1) * NLOC]

    ewl_max = ewh_max = 0
    prepped = []
    for c in range(NC):
        s_c, dloc = core_edges[c]
        s_row = row_of(s_c)
        drow = perms[c][dloc]
        w_of = drow // 128
        is_lo = s_row < HALF
        order = np.lexsort((drow, ~is_lo, w_of))
        s_row, drow, w_of, is_lo = (s_row[order], drow[order],
                                    w_of[order], is_lo[order])
        nlo = np.bincount(w_of[is_lo], minlength=W)
        nhi = np.bincount(w_of[~is_lo], minlength=W)
        ewl_max = max(ewl_max, int(nlo.max()))
        ewh_max = max(ewh_max, int(nhi.max()))
        prepped.append((s_row, drow, w_of, is_lo, nlo, nhi))

    ewl = -(-ewl_max // 128) * 128
    ewh = -(-ewh_max // 128) * 128
    EW = ewl + ewh
    S = EW // 128

    in_maps = []
    shared = dict(w_kv=wkv, w_qs=wqs, b_kv=bkv, b_qs=bqs, ones1=ones1,
                  w_fc=wfc_p, b_fc=bfc_rep.astype(np.float32),
                  iota_in=iota, xT_full=xT,
                  iotaP=np.tile(np.arange(128, dtype=f16)[:, None], (1, EW)))
    for c in range(NC):
        s_row, drow, w_of, is_lo, nlo, nhi = prepped[c]
        ilo = np.zeros((W, ewl), np.int64)
        ihi = np.zeros((W, ewh), np.int64)
        dpos = np.zeros((W, EW), np.int64)
        mpos = np.full((W, EW), MASK_NEG, np.float64)
        wstart = np.searchsorted(w_of, np.arange(W))
        wend = np.searchsorted(w_of, np.arange(W) + 1)
        for w in range(W):
            a, b = int(wstart[w]), int(wend[w])
            k = int(nlo[w])
            ilo[w, :k] = s_row[a:a + k]
            dpos[w, :k] = drow[a:a + k] % 128
            mpos[w, :k] = 0.0
            nh = b - (a + k)
            ihi[w, :nh] = s_row[a + k:b] - HALF
            dpos[w, ewl:ewl + nh] = drow[a + k:b] % 128
            mpos[w, ewl:ewl + nh] = 0.0

        gs_flat = np.zeros((NPAD, G), np.float32)
        orig = np.arange(NLOC)
        gn = batch[c * NLOC + orig]
        gs_flat[perms[c][orig], gn] = inv_counts[gn]
        gsel_in = np.ascontiguousarray(
            gs_flat.reshape(W, 128, G).transpose(1, 0, 2)
            .reshape(128, W * G).astype(f16))

        xT_loc = np.ascontiguousarray(xT[:, c * NPAD:(c + 1) * NPAD])
        in_maps.append(dict(
            shared,
            xT_loc=xT_loc,
            idx_lo=_wrap16(ilo), idx_hi=_wrap16(ihi),
            dstf=_pos128(dpos, f16), maskw=_pos128(mpos, np.float32),
            dstT=np.tile(dpos.reshape(1, W * EW), (128, 1)).astype(f16),
            gsel=gsel_in))
    return in_maps, ewl, ewh, use_bias


def kernel(**inputs):
    global LAST_RESULT
    in_maps, ewl, ewh, use_bias = _host_prep(inputs)
    run_layers = int(os.environ.get("RUN_LAYERS", str(NLAYERS)))
    nc = _build(ewl, ewh, run_layers, use_bias)
    trace = bool(int(os.environ.get("KTRACE", "0")))
    res = bass_utils.run_bass_kernel_spmd(
        nc, in_maps, core_ids=list(range(NC)), trace=trace)
    LAST_RESULT = res
    return res.results[0]["out"].astype(np.float32)


# revision 30
# speedup vs baseline: 2.5677x; 1.6159x over previous
"""Trainium2 Bass kernel for nn_EnhancedGNNTransformerEncoder (4-layer
TransformerConv GNN + mean-pool + linear head).

Sharding: destination nodes (and their incident edges) are split across the
8 NeuronCores; the small weight matrices are replicated.  Per layer each core
computes K/V rows for ALL nodes (cheap matmuls) so the per-edge gathers stay
local, does the segment softmax + weighted aggregation for its own node shard
via one-hot selector matmuls on the PE, and the per-layer hidden state is
exchanged with an AllGather (feature-major fp16).
"""

import os
import sys
import types

import numpy as np

# ---------------------------------------------------------------------------
# NTFF profile hook (absent antenv.axon_hooks on this image) so trace=True
# works under axon.
if "antenv.axon_hooks" not in sys.modules:
    _m = types.ModuleType("antenv.axon_hooks")
    _m._hook = None

    def _set(h):
        _m._hook = h

    def _get():
        return _m._hook

    _m.set_axon_ntff_profile_hook = _set
    _m.get_axon_ntff_profile_hook = _get
    sys.modules["antenv.axon_hooks"] = _m
    try:
        import antenv

        antenv.axon_hooks = _m
    except Exception:
        pass
    try:
        from trn_agent_boot.trn_boot import _ntff_profile_via_ctypes

        _m._hook = _ntff_profile_via_ctypes("/opt/axon/libaxon_pjrt.so")
    except Exception:
        pass

import concourse.bass as bass
import concourse.mybir as mybir
import concourse.tile as tile
from concourse import bacc
from concourse import bass_utils
from concourse.masks import make_identity

F16 = mybir.dt.float16
BF16 = mybir.dt.bfloat16
F32 = mybir.dt.float32
I16 = mybir.dt.int16
AX = mybir.AxisListType
OP = mybir.AluOpType
ACTF = mybir.ActivationFunctionType

# problem constants (hardcoded per the harness contract)
N, E, IN, H, C, G, OUT = 50000, 800000, 128, 8, 32, 64, 64
HC = H * C  # 256
NLAYERS = 4
NC = 8
NLOC = N // NC          # 6250
W = 50                  # windows of 128 dst rows per core
NPAD = W * 128          # 6272 local rows (incl dummies)
NPAD_ALL = NC * NPAD    # 50176 kv rows
HALF = NPAD_ALL // 2    # 25088 (int16-addressable half)
SCALE = float(1.0 / np.sqrt(C))
MASK_NEG = -30000.0

_BUILD_CACHE = {}
LAST_RESULT = None


def _build(ewl, ewh, run_layers, use_bias=True):
    """Build + compile the SPMD program.  ewl/ewh: padded lo/hi edge slots
    per window (multiples of 128)."""
    skip_b = bool(int(os.environ.get("KSKIP_B", "0")))
    skip_c = bool(int(os.environ.get("KSKIP_C", "0")))
    nwin = int(os.environ.get("KWIN", str(W)))
    kops = int(os.environ.get("KOPS", "7"))
    qgather = bool(int(os.environ.get("KQGATHER", "1")))
    key = (ewl, ewh, run_layers, skip_b, skip_c, nwin, kops, use_bias, qgather)
    if key in _BUILD_CACHE:
        return _BUILD_CACHE[key]

    S = (ewl + ewh) // 128          # kv slots per window
    SL, SH = ewl // 128, ewh // 128
    EW = ewl + ewh

    nc = bacc.Bacc("TRN2", target_bir_lowering=False, debug=False,
                   enable_asserts=False, num_devices=NC,
                   num_swdge_queues=4)

    # ---- external inputs (per-core content, same shapes) ----
    xT_loc = nc.dram_tensor("xT_loc", [128, NPAD], F16, kind="ExternalInput")
    w_all = nc.dram_tensor("w_all", [128, NLAYERS, 2, 4 * HC], F16, kind="ExternalInput")
    b_all = nc.dram_tensor("b_all", [1, NLAYERS, 4 * HC], F16, kind="ExternalInput")
    ones1 = nc.dram_tensor("ones1", [1, 128], F16, kind="ExternalInput")
    w_fc = nc.dram_tensor("w_fc", [128, 2, OUT], F16, kind="ExternalInput")
    b_fc = nc.dram_tensor("b_fc", [G, OUT], F32, kind="ExternalInput")
    iota_in = nc.dram_tensor("iota_in", [128, 128], F16, kind="ExternalInput")
    idx_lo = nc.dram_tensor("idx_lo", [128, W * (ewl // 16)], I16, kind="ExternalInput")
    idx_hi = nc.dram_tensor("idx_hi", [128, W * (ewh // 16)], I16, kind="ExternalInput")
    dstf = nc.dram_tensor("dstf", [128, W * S], F16, kind="ExternalInput")
    if qgather:
        idx_q = nc.dram_tensor("idx_q", [128, W * (EW // 16)], I16,
                               kind="ExternalInput")
    else:
        dstT = nc.dram_tensor("dstT", [128, W * EW], F16, kind="ExternalInput")
        iotaP = nc.dram_tensor("iotaP", [128, EW], F16, kind="ExternalInput")
    maskw = nc.dram_tensor("maskw", [128, W * S], F32, kind="ExternalInput")
    gsel = nc.dram_tensor("gsel", [128, W * G], F16, kind="ExternalInput")

    out_d = nc.dram_tensor("out", [G, OUT], F32, kind="ExternalOutput")
    debug = bool(int(os.environ.get("KDEBUG", "0")))
    if debug:
        dbg_h = nc.dram_tensor("dbg_h", [2, 128, NPAD], F16, kind="ExternalOutput")
        dbg_kv = nc.dram_tensor("dbg_kv", [1024, 2 * HC], F16, kind="ExternalOutput")
        dbg_qs = nc.dram_tensor("dbg_qs", [1024, 2 * HC], F16, kind="ExternalOutput")
        dbg_pool = nc.dram_tensor("dbg_pool", [2, 128, OUT], F32, kind="ExternalOutput")

    # ---- internal DRAM ----
    kv_loc = nc.dram_tensor("kv_loc", [NPAD, 2 * HC], F16, kind="Internal")
    kv_full = nc.dram_tensor("kv_full", [NPAD_ALL, 2 * HC], F16, kind="Internal",
                             addr_space="Shared")
    qskip_full = nc.dram_tensor("qskip_full", [NPAD, 2 * HC], F16, kind="Internal")
    hT_shard = nc.dram_tensor("hT_shard", [2, 128, NPAD], F16, kind="Internal")
    pool_part = nc.dram_tensor("pool_part", [2, 128, OUT], F32, kind="Internal")
    pool_sum = nc.dram_tensor("pool_sum", [2, 128, OUT], F32, kind="Internal",
                              addr_space="Shared")

    SLAB = 1280         # 10 node-tiles per slab
    NSLAB = NPAD // SLAB  # 5

    with tile.TileContext(nc) as tc:
        with tc.tile_pool(name="const", bufs=1) as cp, \
             tc.tile_pool(name="slab", bufs=2) as slabp, \
             tc.tile_pool(name="kvb", bufs=2) as kvbp, \
             tc.tile_pool(name="win", bufs=2) as winp, \
             tc.tile_pool(name="psA", bufs=2, space="PSUM") as psA, \
             tc.tile_pool(name="psB", bufs=2, space="PSUM") as psB, \
             tc.tile_pool(name="psT", bufs=2, space="PSUM") as psT, \
             tc.tile_pool(name="psQ", bufs=2, space="PSUM") as psQ:

            # ---- load constants ----
            wall_sb = cp.tile([128, NLAYERS, 2, 4 * HC], F16)
            ball_sb = cp.tile([1, NLAYERS, 4 * HC], F16)
            ones_sb = cp.tile([1, 128], F16)
            wfc_sb = cp.tile([128, 2, OUT], F16)
            bfc_sb = cp.tile([G, OUT], F32)
            iota_sb = cp.tile([128, 128], F16)
            dstf_sb = cp.tile([128, W * S], F16)
            if not qgather:
                iotaP_sb = cp.tile([128, EW], F16)
            maskw_sb = cp.tile([128, W * S], F32)
            gsel_sb = cp.tile([128, W * G], F16)
            ident_sb = cp.tile([128, 128], F16)
            pool_acc = cp.tile([128, 2, OUT], F32)

            for t, d in [(wall_sb, w_all), (ball_sb, b_all),
                         (ones_sb, ones1), (wfc_sb, w_fc),
                         (bfc_sb, b_fc), (iota_sb, iota_in), (dstf_sb, dstf),
                         (maskw_sb, maskw), (gsel_sb, gsel)]:
                nc.sync.dma_start(out=t[:], in_=d.ap())
            if not qgather:
                nc.sync.dma_start(out=iotaP_sb[:], in_=iotaP.ap())
            make_identity(nc, ident_sb[:])

            for layer in range(run_layers):
                KH = 1 if layer == 0 else 2

                # ===== Phase A: [K|V|Q|S] rows for own shard, then kv AllGather =====
                for sl in range(NSLAB):
                    slabs = []
                    for kh in range(KH):
                        st = slabp.tile([128, SLAB], F16, tag="slab", bufs=4)
                        if layer == 0:
                            src_ap = xT_loc.ap()[:, sl * SLAB:(sl + 1) * SLAB]
                        else:
                            src_ap = hT_shard.ap()[kh, :, sl * SLAB:(sl + 1) * SLAB]
                        nc.sync.dma_start(out=st[:], in_=src_ap)
                        slabs.append(st)
                    for half5 in range(2):
                        kvb = kvbp.tile([128, 5, 2 * HC], F16, tag="kvb")
                        qsb = kvbp.tile([128, 5, 2 * HC], F16, tag="qsb")
                        for j5 in range(5):
                            j = half5 * 5 + j5
                            ps = psA.tile([128, 4 * HC], F32, tag="psA")
                            for hf in range(2):
                                cs = slice(hf * 2 * HC, (hf + 1) * 2 * HC)
                                for kh in range(KH):
                                    nc.tensor.matmul(
                                        ps[:, cs],
                                        lhsT=slabs[kh][:, j * 128:(j + 1) * 128],
                                        rhs=wall_sb[:, layer, kh, cs],
                                        start=(kh == 0),
                                        stop=(not use_bias and kh == KH - 1),
                                        skip_group_check=True)
                                if use_bias:
                                    nc.tensor.matmul(
                                        ps[:, cs], lhsT=ones_sb[:],
                                        rhs=ball_sb[:, layer, cs],
                                        start=False, stop=True,
                                        skip_group_check=True)
                            if j % 2 == 0:
                                nc.vector.tensor_copy(kvb[:, j5, :], ps[:, 0:2 * HC])
                                nc.scalar.activation(qsb[:, j5, :],
                                                     ps[:, 2 * HC:4 * HC], ACTF.Copy)
                            else:
                                nc.scalar.activation(kvb[:, j5, :],
                                                     ps[:, 0:2 * HC], ACTF.Copy)
                                nc.vector.tensor_copy(qsb[:, j5, :],
                                                      ps[:, 2 * HC:4 * HC])
                        base = sl * SLAB + half5 * 640
                        dst_ap = kv_loc.ap()[base:base + 640, :]
                        nc.sync.dma_start(
                            out=dst_ap.rearrange("(t p) e -> p t e", p=128),
                            in_=kvb[:])
                        dst_ap = qskip_full.ap()[base:base + 640, :]
                        nc.sync.dma_start(
                            out=dst_ap.rearrange("(t p) e -> p t e", p=128),
                            in_=qsb[:])
                if not skip_c:
                    nc.gpsimd.collective_compute(
                        "AllGather", OP.bypass,
                        replica_groups=[list(range(NC))],
                        ins=[kv_loc.ap()], outs=[kv_full.ap()])
                else:
                    nc.sync.dma_start(
                        out=kv_full.ap()[0:NPAD, :], in_=kv_loc.ap())

                if debug and layer == 0:
                    nc.sync.dma_start(
                        out=dbg_kv.ap().rearrange("(t p) e -> p t e", p=128),
                        in_=kv_full.ap()[:1024, :].rearrange("(t p) e -> p t e", p=128))
                    nc.sync.dma_start(
                        out=dbg_qs.ap().rearrange("(t p) e -> p t e", p=128),
                        in_=qskip_full.ap()[:1024, :].rearrange("(t p) e -> p t e", p=128))

                # ===== Phase B: windows =====
                last = layer == NLAYERS - 1
                if skip_b:
                    continue
                if last:
                    nc.vector.memset(pool_acc[:], 0.0)

                qctr = [0]

                def gather_chunked(out_tile, slot0, in_ap, idx_sb, col0, n,
                                   elem, estep=None):
                    done = 0
                    while done < n:
                        cur = min(1024, n - done)
                        nc.gpsimd.dma_gather(
                            out_ap=out_tile[:, slot0 + done // 128:
                                            slot0 + (done + cur) // 128, :],
                            in_ap=in_ap,
                            idxs_ap=idx_sb[:, col0 + done // 16:
                                           col0 + (done + cur) // 16],
                            num_idxs=cur, num_idxs_reg=cur, elem_size=elem,
                            elem_step=estep, single_packet=True,
                            queue_num=qctr[0] % 4)
                        qctr[0] += 1
                        done += cur

                for w in range(nwin):
                    idxw = winp.tile([128, (ewl + ewh + EW) // 16], I16,
                                     tag="idxw", bufs=3)
                    nc.sync.dma_start(out=idxw[:, 0:ewl // 16],
                                      in_=idx_lo.ap()[:, w * (ewl // 16):
                                                      (w + 1) * (ewl // 16)])
                    nc.sync.dma_start(out=idxw[:, ewl // 16:(ewl + ewh) // 16],
                                      in_=idx_hi.ap()[:, w * (ewh // 16):
                                                      (w + 1) * (ewh // 16)])
                    if qgather:
                        nc.sync.dma_start(
                            out=idxw[:, (ewl + ewh) // 16:],
                            in_=idx_q.ap()[:, w * (EW // 16):(w + 1) * (EW // 16)])
                    kv_t = winp.tile([128, S, 2 * HC], F16, tag="kvt", bufs=3)
                    gather_chunked(kv_t, 0, kv_full.ap()[0:HALF, :],
                                   idxw, 0, ewl, 2 * HC)
                    gather_chunked(kv_t, SL, kv_full.ap()[HALF:NPAD_ALL, :],
                                   idxw, ewl // 16, ewh, 2 * HC)
                    qs_w = winp.tile([128, 2 * HC], F16, tag="qsw", bufs=2)
                    nc.sync.dma_start(
                        out=qs_w[:],
                        in_=qskip_full.ap()[w * 128:(w + 1) * 128, :])
                    skip_w = qs_w[:, HC:2 * HC]
                    q_t = winp.tile([128, S, HC], F16, tag="qt", bufs=2)
                    if qgather:
                        gather_chunked(q_t, 0, qskip_full.ap()[:, 0:HC],
                                       idxw, (ewl + ewh) // 16, EW, HC,
                                       estep=2 * HC)
                    else:
                        dstT_w = winp.tile([128, EW], F16, tag="dstT", bufs=2)
                        nc.sync.dma_start(
                            out=dstT_w[:],
                            in_=dstT.ap()[:, w * EW:(w + 1) * EW])
                        selT = winp.tile([128, S, 128], F16, tag="dxsel",
                                         bufs=4)
                        nc.vector.tensor_tensor(
                            selT[:].rearrange("p s e -> p (s e)"),
                            dstT_w[:], iotaP_sb[:], OP.is_equal)
                        for sq in range(S):
                            qe = psQ.tile([128, HC], F32, tag="qe")
                            nc.tensor.matmul(qe[:], lhsT=selT[:, sq, :],
                                             rhs=qs_w[:, 0:HC],
                                             start=True, stop=True,
                                             skip_group_check=True)
                            nc.scalar.activation(q_t[:, sq, :], qe[:],
                                                 ACTF.Copy)
                    if kops < 2:
                        continue

                    # logits
                    qk = winp.tile([128, S, HC], F16, tag="qkpx", bufs=2)
                    nc.vector.tensor_tensor(qk[:], q_t[:], kv_t[:, :, 0:HC], OP.mult)
                    logits = winp.tile([128, S * H], F32, tag="lg", bufs=2)
                    nc.vector.tensor_reduce(
                        logits[:], qk[:].rearrange("p s (h c) -> p (s h) c", c=C),
                        axis=AX.X, op=OP.add)
                    ml = winp.tile([128, S * H], F32, tag="ml", bufs=2)
                    nc.vector.scalar_tensor_tensor(
                        out=ml[:].rearrange("p (s h) -> p s h", h=H),
                        in0=logits[:].rearrange("p (s h) -> p s h", h=H),
                        scalar=SCALE,
                        in1=maskw_sb[:, w * S:(w + 1) * S, None].to_broadcast(
                            (128, S, H)),
                        op0=OP.mult, op1=OP.add)
                    p_bf = winp.tile([128, S * H], BF16, tag="p", bufs=2)
                    nc.scalar.activation(p_bf[:], ml[:], ACTF.Exp)
                    if kops < 3:
                        continue

                    # wv (+ p columns)
                    pX = winp.tile([128, S, HC], BF16, tag="qkpx", bufs=2)
                    nc.scalar.activation(
                        pX[:].rearrange("p s (h c) -> p s h c", c=C),
                        p_bf[:].rearrange("p (s h) -> p s h", h=H)[:, :, :, None]
                        .to_broadcast((128, S, H, C)),
                        ACTF.Copy)
                    wv = winp.tile([128, S, HC + H], BF16, tag="wv", bufs=2)
                    nc.vector.tensor_tensor(
                        wv[:, :, 0:HC], kv_t[:, :, HC:2 * HC], pX[:], OP.mult)
                    nc.scalar.activation(
                        wv[:, :, HC:HC + H],
                        ml[:].rearrange("p (s h) -> p s h", h=H), ACTF.Exp)

                    if kops < 4:
                        continue
                    # one-hot selector
                    dstX = winp.tile([128, S, 128], F16, tag="dxsel", bufs=4)
                    nc.scalar.activation(
                        dstX[:],
                        dstf_sb[:, w * S:(w + 1) * S, None].to_broadcast(
                            (128, S, 128)),
                        ACTF.Copy)
                    sel = winp.tile([128, S, 128], BF16, tag="dxsel", bufs=4)
                    nc.vector.tensor_tensor(
                        sel[:], dstX[:],
                        iota_sb[:, None, :].to_broadcast((128, S, 128)),
                        OP.is_equal)

                    if kops < 5:
                        continue
                    # aggregate
                    agg = psB.tile([128, HC + H], F32, tag="agg")
                    for s in range(S):
                        nc.tensor.matmul(agg[:], lhsT=sel[:, s, :], rhs=wv[:, s, :],
                                         start=(s == 0), stop=(s == S - 1),
                                         skip_group_check=True)

                    if kops < 6:
                        continue
                    # epilogue
                    rs0 = winp.tile([128, H], F32, tag="rs0", bufs=2)
                    nc.vector.tensor_scalar_add(rs0[:], agg[:, HC:HC + H], 1e-16)
                    rs = winp.tile([128, H], F32, tag="rs", bufs=2)
                    nc.vector.reciprocal(rs[:], rs0[:])
                    tmp = winp.tile([128, HC], F32, tag="tmp", bufs=2)
                    nc.vector.tensor_tensor(
                        tmp[:].rearrange("p (h c) -> p h c", c=C),
                        agg[:, 0:HC].rearrange("p (h c) -> p h c", c=C),
                        rs[:, :, None].to_broadcast((128, H, C)),
                        OP.mult)
                    tmp2 = winp.tile([128, HC], F32, tag="tmp2", bufs=2)
                    nc.vector.tensor_tensor(tmp2[:], tmp[:], skip_w, OP.add)
                    h_nm = winp.tile([128, HC], F16, tag="hnm", bufs=2)
                    nc.scalar.activation(h_nm[:], tmp2[:], ACTF.Relu)

                    if kops < 7:
                        continue
                    if last:
                        for kh in range(2):
                            ptmp = psT.tile([128, OUT], F32, tag="trp",
                                            name="ptmp")
                            nc.tensor.matmul(
                                ptmp[:],
                                lhsT=h_nm[:, kh * 128:(kh + 1) * 128],
                                rhs=gsel_sb[:, w * G:(w + 1) * G],
                                start=True, stop=True,
                                skip_group_check=True)
                            nc.vector.tensor_tensor(
                                pool_acc[:, kh, :], pool_acc[:, kh, :],
                                ptmp[:], OP.add)
                    else:
                        hstage = winp.tile([128, 2, 128], F16, tag="hst", bufs=2)
                        for kh in range(2):
                            trp = psT.tile([128, 128], F16, tag="trp")
                            nc.tensor.transpose(
                                trp[:], h_nm[:, kh * 128:(kh + 1) * 128],
                                ident_sb[:])
                            if kh == 0:
                                nc.vector.tensor_copy(hstage[:, kh, :], trp[:])
                            else:
                                nc.scalar.activation(hstage[:, kh, :], trp[:],
                                                     ACTF.Copy)
                        nc.sync.dma_start(
                            out=hT_shard.ap().rearrange("k p n -> p k n")[
                                :, :, w * 128:(w + 1) * 128],
                            in_=hstage[:])

                # ===== Phase C =====
                if not last:
                    if debug:
                        nc.sync.dma_start(out=dbg_h.ap(), in_=hT_shard.ap())
                else:
                    nc.sync.dma_start(
                        out=pool_part.ap().rearrange("k p o -> p k o"),
                        in_=pool_acc[:])
                    if not skip_c:
                        nc.gpsimd.collective_compute(
                            "AllReduce", OP.add,
                            replica_groups=[list(range(NC))],
                            ins=[pool_part.ap()], outs=[pool_sum.ap()])
                    pooled = cp.tile([128, 2, OUT], F32)
                    nc.sync.dma_start(
                        out=pooled[:],
                        in_=pool_sum.ap().rearrange("k p o -> p k o"))
                    if debug:
                        nc.sync.dma_start(out=dbg_pool.ap(), in_=pool_sum.ap())
                    pooled16 = cp.tile([128, 2, OUT], F16)
                    nc.vector.tensor_copy(pooled16[:], pooled[:])
                    fin = psB.tile([G, OUT], F32, tag="agg")
                    for kh in range(2):
                        nc.tensor.matmul(fin[:], lhsT=pooled16[:, kh, :],
                                         rhs=wfc_sb[:, kh, :],
                                         start=(kh == 0), stop=(kh == 1))
                    out_sb = cp.tile([G, OUT], F32)
                    nc.vector.tensor_tensor(out_sb[:], fin[:], bfc_sb[:], OP.add)
                    nc.sync.dma_start(out=out_d.ap(), in_=out_sb[:])

            if run_layers < NLAYERS:
                # partial build (debug): emit output anyway so run works
                out_sb2 = cp.tile([G, OUT], F32)
                nc.vector.memset(out_sb2[:], 0.0)
                nc.sync.dma_start(out=out_d.ap(), in_=out_sb2[:])

    nc.compile()
    _BUILD_CACHE[key] = nc
    return nc


def _wrap16(a):
    """[W, n] int array -> [128, W*(n//16)] int16 gather-index layout."""
    Wn, n = a.shape
    out = a.reshape(Wn, n // 16, 16).transpose(2, 0, 1).reshape(16, Wn * (n // 16))
    return np.tile(out, (8, 1)).astype(np.int16)


def _pos128(a, dtype):
    """[W, EW] per-position array -> [128, W*S] SBUF layout."""
    Wn, n = a.shape
    return np.ascontiguousarray(
        a.reshape(Wn, n // 128, 128).transpose(2, 0, 1).reshape(
            128, Wn * (n // 128)).astype(dtype))


def _host_prep(inputs):
    x = np.asarray(inputs["x"], np.float32)
    ei = np.asarray(inputs["edge_index"]).astype(np.int64)
    batch = np.asarray(inputs["batch"]).astype(np.int64)
    src, dst = ei[0], ei[1]

    f16 = np.float16
    def pack_w(W0a, W0b, Wla, Wlb):
        w = np.zeros((128, NLAYERS, 2, 2 * HC), f16)
        w[:, 0, 0, 0:HC] = np.asarray(W0a, np.float32).astype(f16)
        w[:, 0, 0, HC:] = np.asarray(W0b, np.float32).astype(f16)
        for l in range(NLAYERS - 1):
            a = np.asarray(Wla[l], np.float32).astype(f16)
            b = np.asarray(Wlb[l], np.float32).astype(f16)
            for kh in range(2):
                w[:, l + 1, kh, 0:HC] = a[kh * 128:(kh + 1) * 128]
                w[:, l + 1, kh, HC:] = b[kh * 128:(kh + 1) * 128]
        return w

    wkv = pack_w(inputs["Wk0"], inputs["Wv0"], inputs["Wk"], inputs["Wv"])
    wqs = pack_w(inputs["Wq0"], inputs["Ws0"], inputs["Wq"], inputs["Ws"])
    wall = np.concatenate([wkv, wqs], axis=3)  # [128, L, 2, 1024]

    def pack_b(b0a, b0b, bla, blb):
        b = np.zeros((1, NLAYERS, 2 * HC), f16)
        b[0, 0, 0:HC] = np.asarray(b0a, np.float32).astype(f16)
        b[0, 0, HC:] = np.asarray(b0b, np.float32).astype(f16)
        for l in range(NLAYERS - 1):
            b[0, l + 1, 0:HC] = np.asarray(bla[l], np.float32).astype(f16)
            b[0, l + 1, HC:] = np.asarray(blb[l], np.float32).astype(f16)
        return b

    bkv = pack_b(inputs["bk0"], inputs["bv0"], inputs["bk"], inputs["bv"])
    bqs = pack_b(inputs["bq0"], inputs["bs0"], inputs["bq"], inputs["bs"])
    ball = np.concatenate([bkv, bqs], axis=2)
    use_bias = bool(np.abs(ball).max() > 0)

    wfc = np.asarray(inputs["Wfc"], np.float32).astype(f16)
    wfc_p = np.ascontiguousarray(wfc.reshape(2, 128, OUT).transpose(1, 0, 2))
    bfc_rep = np.tile(np.asarray(inputs["bfc"], np.float32)[None, :], (G, 1))

    iota = np.tile(np.arange(128, dtype=f16)[None, :], (128, 1))
    ones1 = np.ones((1, 128), f16)

    counts = np.bincount(batch, minlength=G).astype(np.float32)
    inv_counts = (1.0 / np.maximum(counts, 1.0)).astype(np.float32)

    # ---- balanced node->window assignment (per core) ----
    HALF_NODE = HALF  # row threshold
    perms = []        # per core: original local idx -> padded local row
    core_edges = []
    for c in range(NC):
        m = (dst >= c * NLOC) & (dst < (c + 1) * NLOC)
        s_c = src[m]
        dloc = dst[m] - c * NLOC
        # provisional src row (needs perms of src cores; fill later)
        core_edges.append((s_c, dloc))
        lo_e = ((s_c // NLOC) * NPAD + (s_c % NLOC)) < HALF_NODE
        lod = np.bincount(dloc[lo_e], minlength=NLOC).astype(np.int64)
        hid = np.bincount(dloc[~lo_e], minlength=NLOC).astype(np.int64)
        order = np.argsort(-(lod + hid), kind="stable")
        wlo = np.zeros(W); whi = np.zeros(W); wcnt = np.zeros(W, np.int64)
        wof = np.empty(NLOC, np.int64)
        for n in order:
            cand = np.where(wcnt < 128)[0]
            score = np.maximum(wlo[cand] + lod[n], whi[cand] + hid[n])
            j = cand[int(np.argmin(score))]
            wof[n] = j
            wlo[j] += lod[n]; whi[j] += hid[n]; wcnt[j] += 1
        # rank within window
        perm = np.empty(NLOC, np.int64)
        fill = np.zeros(W, np.int64)
        for n in range(NLOC):
            wn = wof[n]
            perm[n] = wn * 128 + fill[wn]
            fill[wn] += 1
        perms.append(perm)

    # src global row with permutation of the owner core
    def row_of(nodes):
        c_of = nodes // NLOC
        r = np.empty(len(nodes), np.int64)
        for c in range(NC):
            mm = c_of == c
            r[mm] = c * NPAD + perms[c][nodes[mm] % NLOC]
        return r

    # -- x transposed, padded + permuted layout --
    xT = np.zeros((128, NPAD_ALL), f16)
    xt = np.ascontiguousarray(x.T.astype(f16))
    for c in range(NC):
        cols = c * NPAD + perms[c]
        xT[:, cols] = xt[:, c * NLOC:(c + 1) * NLOC]

    ewl_max = ewh_max = 0
    prepped = []
    for c in range(NC):
        s_c, dloc = core_edges[c]
        s_row = row_of(s_c)
        drow = perms[c][dloc]
        w_of = drow // 128
        is_lo = s_row < HALF
        order = np.lexsort((drow, ~is_lo, w_of))
        s_row, drow, w_of, is_lo = (s_row[order], drow[order],
                                    w_of[order], is_lo[order])
        nlo = np.bincount(w_of[is_lo], minlength=W)
        nhi = np.bincount(w_of[~is_lo], minlength=W)
        ewl_max = max(ewl_max, int(nlo.max()))
        ewh_max = max(ewh_max, int(nhi.max()))
        prepped.append((s_row, drow, w_of, is_lo, nlo, nhi))

    ewl = -(-ewl_max // 128) * 128
    ewh = -(-ewh_max // 128) * 128
    EW = ewl + ewh
    S = EW // 128

    in_maps = []
    shared = dict(w_all=wall, b_all=ball, ones1=ones1,
                  w_fc=wfc_p, b_fc=bfc_rep.astype(np.float32),
                  iota_in=iota,
                  iotaP=np.tile(np.arange(128, dtype=f16)[:, None], (1, EW)))
    for c in range(NC):
        s_row, drow, w_of, is_lo, nlo, nhi = prepped[c]
        ilo = np.zeros((W, ewl), np.int64)
        ihi = np.zeros((W, ewh), np.int64)
        dpos = np.zeros((W, EW), np.int64)
        mpos = np.full((W, EW), MASK_NEG, np.float64)
        wstart = np.searchsorted(w_of, np.arange(W))
        wend = np.searchsorted(w_of, np.arange(W) + 1)
        for w in range(W):
            a, b = int(wstart[w]), int(wend[w])
            k = int(nlo[w])
            ilo[w, :k] = s_row[a:a + k]
            dpos[w, :k] = drow[a:a + k] % 128
            mpos[w, :k] = 0.0
            nh = b - (a + k)
            ihi[w, :nh] = s_row[a + k:b] - HALF
            dpos[w, ewl:ewl + nh] = drow[a + k:b] % 128
            mpos[w, ewl:ewl + nh] = 0.0

        iq = np.zeros((W, EW), np.int64)
        for w in range(W):
            a, b = int(wstart[w]), int(wend[w])
            k = int(nlo[w])
            iq[w, :k] = drow[a:a + k]
            nh = b - (a + k)
            iq[w, ewl:ewl + nh] = drow[a + k:b]

        gs_flat = np.zeros((NPAD, G), np.float32)
        orig = np.arange(NLOC)
        gn = batch[c * NLOC + orig]
        gs_flat[perms[c][orig], gn] = inv_counts[gn]
        gsel_in = np.ascontiguousarray(
            gs_flat.reshape(W, 128, G).transpose(1, 0, 2)
            .reshape(128, W * G).astype(f16))

        xT_loc = np.ascontiguousarray(xT[:, c * NPAD:(c + 1) * NPAD])
        in_maps.append(dict(
            shared,
            xT_loc=xT_loc,
            idx_lo=_wrap16(ilo), idx_hi=_wrap16(ihi),
            dstf=_pos128(dpos, f16), maskw=_pos128(mpos, np.float32),
            dstT=np.tile(dpos.reshape(1, W * EW), (128, 1)).astype(f16),
            idx_q=_wrap16(iq),
            gsel=gsel_in))
    return in_maps, ewl, ewh, use_bias


def kernel(**inputs):
    global LAST_RESULT
    in_maps, ewl, ewh, use_bias = _host_prep(inputs)
    run_layers = int(os.environ.get("RUN_LAYERS", str(NLAYERS)))
    nc = _build(ewl, ewh, run_layers, use_bias)
    qgather = bool(int(os.environ.get("KQGATHER", "1")))
    drop = ("dstT", "iotaP") if qgather else ("idx_q",)
    in_maps = [{k: v for k, v in m.items() if k not in drop} for m in in_maps]
    trace = bool(int(os.environ.get("KTRACE", "0")))
    res = bass_utils.run_bass_kernel_spmd(
        nc, in_maps, core_ids=list(range(NC)), trace=trace)
    LAST_RESULT = res
    return res.results[0]["out"].astype(np.float32)
